# revision 4
# baseline (speedup 1.0000x reference)
"""Trainium2 Bass kernel for an FFM (field-aware factorization machine) forward pass.

Reference computation (all fp32):
    12 embedding matmuls over column slices of fv [32768, 2668], 15 pairwise
    dot-product cross terms, a linear layer and a sigmoid.

The kernel is DMA-bound: fv is 22 MB/core in fp16 (~66 us at effective HBM
bandwidth), so the design pushes every engine's busy time below the DMA
floor (measured per-op costs for a [128,512] tile: PE matmul 213ns, DVE
TSS 194 / TT 327 / STT ~750, Pool TT ~990 (Pool TSS/STT are unusable:
8us / unsupported), ACT copy ~500):

  * The 12 embeddings are packed as 64-row halves of 128-row weight blocks:
      A = [uu | ui+S3]   tiles 0..7 + an S3 matmul on tile 20
      B = [ti | tu+mu]   tiles 7..20
      D = [mi | au+gu],  E = [R | au+ou]   (tile 20 only)
    where R = au+gu+ou, S3 = ai+gi+oi. Since the cross terms contain
    (tu+mu)*(ui+S3), accumulating S3 into psA's high half merges two
    products into one. 25 block matmuls + 1 reduce matmul per sub.
  * Cross terms are 4 tensor_tensor half-products on drained fp16 tiles:
    uu*R, (ui+S3)*(tu+mu), mi*ti, (au+gu)*(au+ou); the -au*au correction
    rides the linear chain via a host-derived fv row holding fv_2626^2
    with linear weight -||A_u||^2. Most products + the fold adds run on
    the otherwise idle Pool engine; one ones-matmul closes the logit.
  * The linear term is split: some K-tiles as M=1 fp16 PE matmuls into
    the logit PSUM (grouped after the blocks), the rest as DVE
    TSS-multiply + TT-add pairs (cheaper than the fused STT on HW),
    optionally a few as ACT copies with a per-partition scale.
  * PSUM is drained to fp16 SBUF on the ACT engine; all DVE/Pool operands
    are then 2-byte + SBUF-only, enabling the DVE fast modes.
  * Sub n's products and reduce retire one sub late (software pipelining)
    so the in-order engine queues never stall on the previous sub's tail.
  * fv is streamed sub-major: one contiguous [128, 21*512] fp16 DMA per
    512-batch sub (host pre-arranges the layout), split across two HWDGE
    rings for bandwidth.

Distribution: data-parallel over the batch dim - each of the 8 cores gets
4096 rows, cast to fp16 host-side (halves HBM traffic; rel err ~4e-3 vs
the 2e-2 gate).
"""

import os
import numpy as np
from contextlib import ExitStack

B, F, D = 32768, 2668, 64
NCORES = 8
BL = B // NCORES          # batch rows per core
NKT = 21                  # feature K-tiles of 128
FP = NKT * 128            # padded feature dim (2688)
NSUB = 512                # matmul moving-dim (one fp32 PSUM bank)
NSUBS = BL // NSUB        # subs per core (8)
SW = NKT * NSUB           # packed width of one sub (10752)

# w_pack column layout: A t0..7 | B t7..20 | S3 | D | E  (128 cols per tile)
A_TILES = tuple(range(0, 8))
B_TILES = tuple(range(7, 21))
AOFF = {t: i * 128 for i, t in enumerate(A_TILES)}
BOFF = {t: (8 + i) * 128 for i, t in enumerate(B_TILES)}
S3OFF, DOFF, EOFF = 22 * 128, 23 * 128, 24 * 128
WF = 25 * 128

# linear-term K-tile split across engines (tunable)
PE_LIN = int(os.environ.get("FFM_PE_LIN", "9"))
ACT_LIN = int(os.environ.get("FFM_ACT_LIN", "0"))


def _lin_split():
    """Deterministic partition of the 21 K-tiles among PE/ACT/DVE."""
    order = list(range(NKT))
    pe_t = order[0::2][:PE_LIN]
    rest = [t for t in order if t not in pe_t]
    act_t = rest[0::2][:ACT_LIN]
    dve_t = [t for t in rest if t not in act_t]
    return tuple(pe_t), set(act_t), set(dve_t)


PE_T, ACT_T, DVE_T = _lin_split()


def _build_w_pack(inp):
    """Pack the block tables into one [128, WF] array laid out exactly as
    the SBUF weight tile wants it (partition k = row-in-K-tile)."""
    A_u, A_i = inp["age_user_w"], inp["age_item_w"]
    G_u, G_i = inp["gender_user_w"], inp["gender_item_w"]
    O_u, O_i = inp["occupation_user_w"], inp["occupation_item_w"]
    M_u, M_i = inp["movie_user_w"], inp["movie_item_w"]
    U_u, U_i = inp["userid_user_w"], inp["userid_item_w"]
    T_u, T_i = inp["itemid_user_w"], inp["itemid_item_w"]

    WA = np.zeros((FP, 128), np.float32)
    WA[0:943, 0:64] = U_u; WA[0:943, 64:128] = U_i
    WB = np.zeros((FP, 128), np.float32)
    WB[943:2625, 0:64] = T_i
    WB[943:2625, 64:128] = T_u; WB[2649:2668, 64:128] = M_u
    WS3 = np.zeros((FP, 128), np.float32)
    WS3[2626:2627, 64:128] += A_i; WS3[2626:2628, 64:128] += G_i
    WS3[2628:2649, 64:128] += O_i
    WD = np.zeros((FP, 128), np.float32)
    WD[2649:2668, 0:64] = M_i
    WD[2626:2627, 64:128] += A_u; WD[2626:2628, 64:128] += G_u
    WE = np.zeros((FP, 128), np.float32)
    WE[2626:2627, 0:64] += A_u; WE[2626:2628, 0:64] += G_u
    WE[2628:2649, 0:64] += O_u
    WE[2626:2627, 64:128] += A_u; WE[2628:2649, 64:128] += O_u

    w_pack = np.zeros((128, WF), np.float32)
    for t in A_TILES:
        w_pack[:, AOFF[t]:AOFF[t] + 128] = WA[t * 128:(t + 1) * 128]
    for t in B_TILES:
        w_pack[:, BOFF[t]:BOFF[t] + 128] = WB[t * 128:(t + 1) * 128]
    w_pack[:, S3OFF:S3OFF + 128] = WS3[20 * 128:21 * 128]
    w_pack[:, DOFF:DOFF + 128] = WD[20 * 128:21 * 128]
    w_pack[:, EOFF:EOFF + 128] = WE[20 * 128:21 * 128]
    return w_pack


def _trace_kernel(ctx: ExitStack, tc, out_d, fvt_d, w_d, lin_d, lb_d,
                  ones_d, repeat=1, loop=False):
    import concourse.mybir as mybir

    nc = tc.nc
    f32 = mybir.dt.float32
    f16 = mybir.dt.float16
    MUL = mybir.AluOpType.mult
    ADD = mybir.AluOpType.add
    COPY = mybir.ActivationFunctionType.Copy

    wpool = ctx.enter_context(tc.tile_pool(name="wpool", bufs=1))
    w_sb = wpool.tile([128, WF], f16, name="w_sb")
    nc.sync.dma_start(w_sb[:, 0:128], w_d[:, 0:128])
    nc.sync.dma_start(w_sb[:, 128:WF], w_d[:, 128:WF])
    lin_sb = wpool.tile([128, NKT], f32, name="lin_sb")
    nc.sync.dma_start(lin_sb[:], lin_d[:])
    lin16_sb = wpool.tile([128, NKT], f16, name="lin16_sb")
    nc.gpsimd.dma_start(lin16_sb[:], lin_d[:])  # casting DMA (f32 -> f16)
    lb_sb = wpool.tile([1, 1], f32, name="lb_sb")
    nc.sync.dma_start(lb_sb[:], lb_d[:])
    ones_sb = wpool.tile([128, 1], f16, name="ones_sb")
    nc.sync.dma_start(ones_sb[:], ones_d[:])

    fpool = ctx.enter_context(tc.tile_pool(
        name="fpool", bufs=int(os.environ.get("FFM_FBUFS", "3"))))
    pspool = ctx.enter_context(tc.tile_pool(name="pspool", bufs=1, space="PSUM"))
    spool = ctx.enter_context(tc.tile_pool(name="spool", bufs=3))
    opool = ctx.enter_context(tc.tile_pool(name="opool", bufs=2))
    out_eng = {"sync": nc.sync, "scalar": nc.scalar,
               "gpsimd": nc.gpsimd}[os.environ.get("FFM_OUTDMA", "sync")]
    lin_stt = os.environ.get("FFM_LIN_STT", "0") == "1"
    pool_prod = int(os.environ.get("FFM_POOL_PROD", "3"))
    pool_folds = int(os.environ.get("FFM_POOL_FOLDS", "2"))
    reduces = int(os.environ.get("FFM_REDUCES", "1"))

    HALF = SW // 2  # ring-split point of the packed sub row

    def _products(p):
        """Cross products + folds for a sub whose drains landed a sub ago."""
        sid = p["sid"]
        dA, dB, dD, dE = p["dA"], p["dB"], p["dD"], p["dE"]
        st1 = spool.tile([128, NSUB], f16, tag="st1", name=f"st1_{sid}")
        st2 = spool.tile([128, NSUB], f16, tag="st2", name=f"st2_{sid}")
        prods = [
            (st1, slice(0, 64), dA, dE),        # uu*R
            (st1, slice(64, 128), dA, dB),      # (ui+S3)*(tu+mu)
            (st2, slice(0, 64), dD, dB),        # mi*ti
            (st2, slice(64, 128), dD, dE),      # (au+gu)*(au+ou) [-au^2 in lin]
        ]
        for j, (dst, sl, x, y) in enumerate(prods):
            eng = nc.gpsimd if j < pool_prod else nc.vector
            eng.tensor_mul(dst[sl], x[sl], y[sl])
        # fold ACT-lin tmp tiles into the DVE accumulators
        accs = [a for a in (p["acc0"], p["acc1"]) if a is not None]
        srcs = [st1, st2] + accs
        for i, tmp in enumerate(p["atmps"]):
            if accs:
                tgt = accs[i % len(accs)]
                nc.vector.tensor_add(tgt[:], tgt[:], tmp[:])
            else:
                srcs.append(tmp)
        # fold source pairs so fewer PE reduce matmuls are needed
        nf = 0
        while len(srcs) > max(reduces, 1):
            a = srcs.pop(0); b = srcs.pop(0)
            eng = nc.gpsimd if nf < pool_folds else nc.vector
            eng.tensor_add(a[:], a[:], b[:])
            srcs.append(a)
            nf += 1
        p["red"] = tuple(srcs)

    def _finish(p):
        """Reduce + sigmoid + store for a sub whose products are complete."""
        red, col, logit = p["red"], p["col"], p["logit"]
        for j, srct in enumerate(red):
            nc.tensor.matmul(logit[:], ones_sb[:], srct[:],
                             start=(j == 0 and not PE_T),
                             stop=(j == len(red) - 1))
        out_sb = opool.tile([1, NSUB], f32, tag="out", name=f"out_{col}")
        nc.scalar.activation(out_sb[:], logit[:],
                             mybir.ActivationFunctionType.Sigmoid,
                             bias=lb_sb[0:1, 0:1], scale=1.0)
        out_eng.dma_start(out_d[0:1, col:col + NSUB], out_sb[:])

    def _body(rep, passes=1):
        pend = []  # software-pipeline: sub n retires during sub n+1
        for sp in range(passes * NSUBS):
            s = sp % NSUBS
            big = fpool.tile([128, SW], f16, tag="fv", name=f"fv_{rep}_{sp}")
            nc.sync.dma_start(big[:, 0:HALF], fvt_d[s][:, 0:HALF])
            nc.scalar.dma_start(big[:, HALF:SW], fvt_d[s][:, HALF:SW])

            def rhs(t):
                return big[:, t * NSUB:(t + 1) * NSUB]
            sid = f"{rep}_{sp}"
            psA = pspool.tile([128, NSUB], f32, tag="psA", bufs=2,
                              name=f"psA_{sid}")
            psB = pspool.tile([128, NSUB], f32, tag="psB", bufs=2,
                              name=f"psB_{sid}")
            logit = pspool.tile([1, NSUB], f32, tag="logit", bufs=2,
                                name=f"logit_{sid}")
            psD = pspool.tile([128, NSUB], f32, tag="psD", name=f"psD_{sid}")
            psE = pspool.tile([128, NSUB], f32, tag="psE", name=f"psE_{sid}")
            accs = [None, None]   # DVE parities (hide RAW latency)
            atmps = []            # ACT-lin tmp tiles
            ndve = 0
            for t in range(NKT):
                if t in A_TILES:
                    nc.tensor.matmul(psA[:], w_sb[:, AOFF[t]:AOFF[t] + 128],
                                     rhs(t), start=(t == 0), stop=False)
                if t in B_TILES:
                    nc.tensor.matmul(psB[:], w_sb[:, BOFF[t]:BOFF[t] + 128],
                                     rhs(t), start=(t == 7), stop=(t == 20))
                if t == 20:
                    nc.tensor.matmul(psA[:], w_sb[:, S3OFF:S3OFF + 128],
                                     rhs(t), start=False, stop=True)
                    nc.tensor.matmul(psD[:], w_sb[:, DOFF:DOFF + 128],
                                     rhs(t), start=True, stop=True)
                    nc.tensor.matmul(psE[:], w_sb[:, EOFF:EOFF + 128],
                                     rhs(t), start=True, stop=True)
                # linear term: per-partition-scalar multiply-accumulate,
                # split across PE (M=1 matmuls grouped after the blocks),
                # ACT (scaled copies) and DVE (TSS+add pairs)
                wcol = lin_sb[:, t:t + 1]
                if t in PE_T:
                    pass  # emitted below, grouped with the reduce
                elif t in ACT_T:
                    at = spool.tile([128, NSUB], f16, tag=f"atmp{len(atmps)}",
                                    name=f"atmp{len(atmps)}_{sid}")
                    nc.scalar.activation(at[:], rhs(t), COPY, scale=wcol)
                    atmps.append(at)
                else:
                    par = ndve % 2
                    ndve += 1
                    if accs[par] is None:
                        at = spool.tile([128, NSUB], f16, tag=f"acc{par}",
                                        name=f"acc{par}_{sid}")
                        nc.vector.tensor_single_scalar(at[:], rhs(t),
                                                       wcol, MUL)
                        accs[par] = at
                    elif lin_stt:
                        nc.vector.scalar_tensor_tensor(
                            accs[par][:], rhs(t), wcol, accs[par][:],
                            MUL, ADD)
                    else:
                        tmp = spool.tile([128, NSUB], f16, tag=f"tmp{par}",
                                         name=f"tmp{par}_{sid}_{t}")
                        nc.vector.tensor_single_scalar(tmp[:], rhs(t),
                                                       wcol, MUL)
                        nc.vector.tensor_add(accs[par][:], accs[par][:],
                                             tmp[:])
            # linear-term M=1 matmuls, grouped so the PE switches from
            # 128-row to 1-row output tiles only once per sub
            for j, t in enumerate(PE_T):
                nc.tensor.matmul(logit[:], lin16_sb[:, t:t + 1],
                                 rhs(t), start=(j == 0), stop=False)
            # drains: PSUM -> fp16 SBUF on the ACT engine, in PE completion
            # order (B stops first, then A/D/E at t20)
            dB = spool.tile([128, NSUB], f16, tag="dB", name=f"dB_{sid}")
            nc.scalar.copy(dB[:], psB[:])
            dA = spool.tile([128, NSUB], f16, tag="dA", name=f"dA_{sid}")
            nc.scalar.copy(dA[:], psA[:])
            dD = spool.tile([128, NSUB], f16, tag="dD", name=f"dD_{sid}")
            nc.scalar.copy(dD[:], psD[:])
            dE = spool.tile([128, NSUB], f16, tag="dE", name=f"dE_{sid}")
            nc.scalar.copy(dE[:], psE[:])
            # retire older subs: products + reduce one sub behind
            depth = int(os.environ.get("FFM_DEPTH", "1"))
            if pend:
                _products(pend[-1])
            while len(pend) >= depth:
                _finish(pend.pop(0))
            pend.append({"sid": sid, "dA": dA, "dB": dB, "dD": dD,
                         "dE": dE, "acc0": accs[0], "acc1": accs[1],
                         "atmps": atmps, "logit": logit, "col": s * NSUB})
        _products(pend[-1])
        for p in pend:
            _finish(p)

    if loop and repeat > 1:
        # benchmarking mode: run the identical body `repeat` times inside one
        # NEFF via a hardware loop, multiple passes per iteration so the
        # software pipeline flows across pass boundaries.
        if repeat % 16 == 0:
            with tc.For_i(0, repeat // 16, 1):
                _body(0, passes=16)
        elif repeat % 8 == 0:
            with tc.For_i(0, repeat // 8, 1):
                _body(0, passes=8)
        elif repeat % 4 == 0:
            with tc.For_i(0, repeat // 4, 1):
                _body(0, passes=4)
        elif repeat % 2 == 0:
            with tc.For_i(0, repeat // 2, 1):
                _body(0, passes=2)
        else:
            with tc.For_i(0, repeat, 1):
                _body(0)
    else:
        for rep in range(repeat):
            _body(rep)


_MODULES = {}


def get_module(repeat=1, loop=False):
    """Build (once per config) and return the compiled Bass module."""
    key = (repeat, loop)
    if key in _MODULES:
        return _MODULES[key]

    import concourse.bacc as bacc
    import concourse.tile as tile
    import concourse.mybir as mybir

    nc = bacc.Bacc("TRN2", debug=False, enable_asserts=False,
                   num_devices=NCORES)
    fvt_d = nc.dram_tensor("fvt", (NSUBS, 128, SW), mybir.dt.float16,
                           kind="ExternalInput").ap()
    w_d = nc.dram_tensor("wpack", (128, WF), mybir.dt.float16,
                         kind="ExternalInput").ap()
    lin_d = nc.dram_tensor("lin32", (128, NKT), mybir.dt.float32,
                           kind="ExternalInput").ap()
    lb_d = nc.dram_tensor("linb", (1, 1), mybir.dt.float32,
                          kind="ExternalInput").ap()
    ones_d = nc.dram_tensor("ones16", (128, 1), mybir.dt.float16,
                            kind="ExternalInput").ap()
    out_d = nc.dram_tensor("out", (1, BL), mybir.dt.float32,
                           kind="ExternalOutput").ap()

    with tile.TileContext(nc) as tc, ExitStack() as ctx:
        _trace_kernel(ctx, tc, out_d, fvt_d, w_d, lin_d, lb_d,
                      ones_d, repeat=repeat, loop=loop)
    nc.compile()
    _MODULES[key] = nc
    return nc


def prepare_in_maps(inputs):
    """Host-side sharding: batch-split fv, pack each shard sub-major as
    [NSUBS, 128, NKT*512] fp16 (one contiguous DMA per sub), replicate
    the packed weights."""
    fv = np.ascontiguousarray(np.asarray(inputs["feature_vector"], np.float32))
    assert fv.shape == (B, F)
    tables = {k: np.asarray(v, np.float32) for k, v in inputs.items()
              if k != "feature_vector"}
    w_pack = np.ascontiguousarray(_build_w_pack(tables), np.float16)
    lw = np.zeros(FP, np.float32)
    lw[:F] = tables["lin_w"][0]
    # -au*au correction rides the linear chain: a derived fv row holds
    # fv_2626^2 and its linear weight is -||A_u||^2
    lw[F + 1] = -float((tables["age_user_w"][0] ** 2).sum())
    lin32 = np.ascontiguousarray(lw.reshape(NKT, 128).T)
    lb = tables["lin_b"].reshape(1, 1)
    ones16 = np.ones((128, 1), np.float16)

    in_maps = []
    for c in range(NCORES):
        fvt = np.zeros((FP, BL), np.float16)
        fvt[:F] = fv[c * BL:(c + 1) * BL].T
        fvt[F + 1] = fv[c * BL:(c + 1) * BL, 2626] ** 2
        # [t*128+p, s*512+c] -> [s, p, t*512+c]
        fvt = np.ascontiguousarray(
            fvt.reshape(NKT, 128, NSUBS, NSUB).transpose(2, 1, 0, 3)
               .reshape(NSUBS, 128, SW))
        in_maps.append({"fvt": fvt, "wpack": w_pack, "lin32": lin32,
                        "linb": lb, "ones16": ones16})
    return in_maps


def kernel(**inputs) -> np.ndarray:
    # Tracing needs the axon NTFF hook, which this environment lacks; make
    # sure a stray BASS_TRACE=1 can't crash the run.
    os.environ["BASS_NEVER_TRACE"] = "1"
    from concourse import bass_utils

    in_maps = prepare_in_maps(inputs)
    nc = get_module()
    try:
        res = bass_utils.run_bass_kernel_spmd(nc, in_maps,
                                              core_ids=list(range(NCORES)))
    except Exception:
        # transient NRT device errors have been observed on this fabric;
        # one retry after a short pause usually succeeds
        import time
        time.sleep(15)
        res = bass_utils.run_bass_kernel_spmd(nc, in_maps,
                                              core_ids=list(range(NCORES)))
    out = np.concatenate([r["out"].reshape(BL) for r in res.results])
    return out.reshape(B, 1).astype(np.float32)


# revision 9
# speedup vs baseline: 1.0401x; 1.0401x over previous
"""Trainium2 Bass kernel for an FFM (field-aware factorization machine) forward pass.

Reference computation (all fp32):
    12 embedding matmuls over column slices of fv [32768, 2668], 15 pairwise
    dot-product cross terms, a linear layer and a sigmoid.

The kernel is DMA-bound: fv is 22 MB/core in fp16 (~66 us at effective HBM
bandwidth), so the design pushes every engine's busy time below the DMA
floor (measured per-op costs for a [128,512] tile: PE matmul 213ns, DVE
TSS 194 / TT 327 / STT ~750, Pool TT ~990 (Pool TSS/STT are unusable:
8us / unsupported), ACT copy ~500):

  * The 12 embeddings are packed as 64-row halves of 128-row weight blocks:
      A = [uu | ui+S3]   tiles 0..7 + an S3 matmul on tile 20
      B = [ti | tu+mu]   tiles 7..20
      D = [mi | au+gu],  E = [R | au+ou]   (tile 20 only)
    where R = au+gu+ou, S3 = ai+gi+oi. Since the cross terms contain
    (tu+mu)*(ui+S3), accumulating S3 into psA's high half merges two
    products into one. 25 block matmuls + 1 reduce matmul per sub.
  * Cross terms are 4 tensor_tensor half-products on drained fp16 tiles:
    uu*R, (ui+S3)*(tu+mu), mi*ti, (au+gu)*(au+ou); the -au*au correction
    rides the linear chain via a host-derived fv row holding fv_2626^2
    with linear weight -||A_u||^2. Most products + the fold adds run on
    the otherwise idle Pool engine; one ones-matmul closes the logit.
  * The linear term is split: some K-tiles as M=1 fp16 PE matmuls into
    the logit PSUM (grouped after the blocks), the rest as DVE
    TSS-multiply + TT-add pairs (cheaper than the fused STT on HW),
    optionally a few as ACT copies with a per-partition scale.
  * PSUM is drained to fp16 SBUF on the ACT engine; all DVE/Pool operands
    are then 2-byte + SBUF-only, enabling the DVE fast modes.
  * Sub n's products and reduce retire one sub late (software pipelining)
    so the in-order engine queues never stall on the previous sub's tail.
  * fv is streamed sub-major: one contiguous [128, 21*512] fp16 DMA per
    512-batch sub (host pre-arranges the layout), split across two HWDGE
    rings for bandwidth.

Distribution: data-parallel over the batch dim - each of the 8 cores gets
4096 rows, cast to fp16 host-side (halves HBM traffic; rel err ~4e-3 vs
the 2e-2 gate).
"""

import os
import numpy as np
from contextlib import ExitStack

B, F, D = 32768, 2668, 64
NCORES = 8
BL = B // NCORES          # batch rows per core
NKT = 21                  # feature K-tiles of 128
FP = NKT * 128            # padded feature dim (2688)
NSUB = 512                # matmul moving-dim (one fp32 PSUM bank)
NSUBS = BL // NSUB        # subs per core (8)
SW = NKT * NSUB           # packed width of one sub (10752)

# w_pack column layout: A t0..7 | B t7..20 | S3 | D | E  (128 cols per tile)
A_TILES = tuple(range(0, 8))
B_TILES = tuple(range(7, 21))
AOFF = {t: i * 128 for i, t in enumerate(A_TILES)}
BOFF = {t: (8 + i) * 128 for i, t in enumerate(B_TILES)}
S3OFF, DOFF, EOFF = 22 * 128, 23 * 128, 24 * 128
WF = 25 * 128

# linear-term K-tile split across engines (tunable)
PE_LIN = int(os.environ.get("FFM_PE_LIN", "6"))
ACT_LIN = int(os.environ.get("FFM_ACT_LIN", "3"))


def _lin_split():
    """Deterministic partition of the 21 K-tiles among PE/ACT/DVE."""
    order = list(range(NKT))
    pe_t = order[0::2][:PE_LIN]
    rest = [t for t in order if t not in pe_t]
    act_t = rest[0::2][:ACT_LIN]
    dve_t = [t for t in rest if t not in act_t]
    return tuple(pe_t), set(act_t), set(dve_t)


PE_T, ACT_T, DVE_T = _lin_split()


def _build_w_pack(inp):
    """Pack the block tables into one [128, WF] array laid out exactly as
    the SBUF weight tile wants it (partition k = row-in-K-tile)."""
    A_u, A_i = inp["age_user_w"], inp["age_item_w"]
    G_u, G_i = inp["gender_user_w"], inp["gender_item_w"]
    O_u, O_i = inp["occupation_user_w"], inp["occupation_item_w"]
    M_u, M_i = inp["movie_user_w"], inp["movie_item_w"]
    U_u, U_i = inp["userid_user_w"], inp["userid_item_w"]
    T_u, T_i = inp["itemid_user_w"], inp["itemid_item_w"]

    WA = np.zeros((FP, 128), np.float32)
    WA[0:943, 0:64] = U_u; WA[0:943, 64:128] = U_i
    WB = np.zeros((FP, 128), np.float32)
    WB[943:2625, 0:64] = T_i
    WB[943:2625, 64:128] = T_u; WB[2649:2668, 64:128] = M_u
    WS3 = np.zeros((FP, 128), np.float32)
    WS3[2626:2627, 64:128] += A_i; WS3[2626:2628, 64:128] += G_i
    WS3[2628:2649, 64:128] += O_i
    WD = np.zeros((FP, 128), np.float32)
    WD[2649:2668, 0:64] = M_i
    WD[2626:2627, 64:128] += A_u; WD[2626:2628, 64:128] += G_u
    WE = np.zeros((FP, 128), np.float32)
    WE[2626:2627, 0:64] += A_u; WE[2626:2628, 0:64] += G_u
    WE[2628:2649, 0:64] += O_u
    WE[2626:2627, 64:128] += A_u; WE[2628:2649, 64:128] += O_u

    w_pack = np.zeros((128, WF), np.float32)
    for t in A_TILES:
        w_pack[:, AOFF[t]:AOFF[t] + 128] = WA[t * 128:(t + 1) * 128]
    for t in B_TILES:
        w_pack[:, BOFF[t]:BOFF[t] + 128] = WB[t * 128:(t + 1) * 128]
    w_pack[:, S3OFF:S3OFF + 128] = WS3[20 * 128:21 * 128]
    w_pack[:, DOFF:DOFF + 128] = WD[20 * 128:21 * 128]
    w_pack[:, EOFF:EOFF + 128] = WE[20 * 128:21 * 128]
    return w_pack


def _trace_kernel(ctx: ExitStack, tc, out_d, fvt_d, w_d, lin_d, lb_d,
                  ones_d, repeat=1, loop=False):
    import concourse.mybir as mybir

    nc = tc.nc
    f32 = mybir.dt.float32
    f16 = mybir.dt.float16
    MUL = mybir.AluOpType.mult
    ADD = mybir.AluOpType.add
    COPY = mybir.ActivationFunctionType.Copy

    wpool = ctx.enter_context(tc.tile_pool(name="wpool", bufs=1))
    w_sb = wpool.tile([128, WF], f16, name="w_sb")
    nc.sync.dma_start(w_sb[:, 0:128], w_d[:, 0:128])
    nc.sync.dma_start(w_sb[:, 128:WF], w_d[:, 128:WF])
    lin_sb = wpool.tile([128, NKT], f32, name="lin_sb")
    nc.sync.dma_start(lin_sb[:], lin_d[:])
    lin16_sb = wpool.tile([128, NKT], f16, name="lin16_sb")
    nc.gpsimd.dma_start(lin16_sb[:], lin_d[:])  # casting DMA (f32 -> f16)
    lb_sb = wpool.tile([1, 1], f32, name="lb_sb")
    nc.sync.dma_start(lb_sb[:], lb_d[:])
    ones_sb = wpool.tile([128, 1], f16, name="ones_sb")
    nc.sync.dma_start(ones_sb[:], ones_d[:])

    fpool = ctx.enter_context(tc.tile_pool(
        name="fpool", bufs=int(os.environ.get("FFM_FBUFS", "4"))))
    pspool = ctx.enter_context(tc.tile_pool(name="pspool", bufs=1, space="PSUM"))
    spool = ctx.enter_context(tc.tile_pool(name="spool", bufs=3))
    opool = ctx.enter_context(tc.tile_pool(name="opool", bufs=2))
    out_eng = {"sync": nc.sync, "scalar": nc.scalar,
               "gpsimd": nc.gpsimd}[os.environ.get("FFM_OUTDMA", "sync")]
    lin_stt = os.environ.get("FFM_LIN_STT", "0") == "1"
    pool_prod = int(os.environ.get("FFM_POOL_PROD", "0"))
    pool_folds = int(os.environ.get("FFM_POOL_FOLDS", "0"))
    reduces = int(os.environ.get("FFM_REDUCES", "1"))

    HALF = SW // 2  # ring-split point of the packed sub row

    def _products(p):
        """Cross products + folds for a sub whose drains landed a sub ago."""
        sid = p["sid"]
        dA, dB, dD, dE = p["dA"], p["dB"], p["dD"], p["dE"]
        st1 = spool.tile([128, NSUB], f16, tag="st1", name=f"st1_{sid}")
        st2 = spool.tile([128, NSUB], f16, tag="st2", name=f"st2_{sid}")
        prods = [
            (st1, slice(0, 64), dA, dE),        # uu*R
            (st1, slice(64, 128), dA, dB),      # (ui+S3)*(tu+mu)
            (st2, slice(0, 64), dD, dB),        # mi*ti
            (st2, slice(64, 128), dD, dE),      # (au+gu)*(au+ou) [-au^2 in lin]
        ]
        for j, (dst, sl, x, y) in enumerate(prods):
            eng = nc.gpsimd if j < pool_prod else nc.vector
            eng.tensor_mul(dst[sl], x[sl], y[sl])
        # fold ACT-lin tmp tiles into the DVE accumulators
        accs = [a for a in (p["acc0"], p["acc1"]) if a is not None]
        srcs = [st1, st2] + accs
        for i, tmp in enumerate(p["atmps"]):
            if accs:
                tgt = accs[i % len(accs)]
                nc.vector.tensor_add(tgt[:], tgt[:], tmp[:])
            else:
                srcs.append(tmp)
        # fold source pairs so fewer PE reduce matmuls are needed
        nf = 0
        while len(srcs) > max(reduces, 1):
            a = srcs.pop(0); b = srcs.pop(0)
            eng = nc.gpsimd if nf < pool_folds else nc.vector
            eng.tensor_add(a[:], a[:], b[:])
            srcs.append(a)
            nf += 1
        p["red"] = tuple(srcs)

    def _finish(p):
        """Reduce + sigmoid + store for a sub whose products are complete."""
        red, col, logit = p["red"], p["col"], p["logit"]
        for j, srct in enumerate(red):
            nc.tensor.matmul(logit[:], ones_sb[:], srct[:],
                             start=(j == 0 and not PE_T),
                             stop=(j == len(red) - 1))
        out_sb = opool.tile([1, NSUB], f32, tag="out", name=f"out_{col}")
        nc.scalar.activation(out_sb[:], logit[:],
                             mybir.ActivationFunctionType.Sigmoid,
                             bias=lb_sb[0:1, 0:1], scale=1.0)
        out_eng.dma_start(out_d[0:1, col:col + NSUB], out_sb[:])

    def _body(rep, passes=1):
        pend = []  # software-pipeline: sub n retires during sub n+1
        for sp in range(passes * NSUBS):
            s = sp % NSUBS
            big = fpool.tile([128, SW], f16, tag="fv", name=f"fv_{rep}_{sp}")
            nc.sync.dma_start(big[:, 0:HALF], fvt_d[s][:, 0:HALF])
            nc.scalar.dma_start(big[:, HALF:SW], fvt_d[s][:, HALF:SW])
            # retire path of the previous sub FIRST: its products/folds land
            # at the head of the in-order DVE queue (its drains are already
            # done), so the PE reduce emitted at this sub's tail never stalls
            if pend:
                _products(pend[-1])

            def rhs(t):
                return big[:, t * NSUB:(t + 1) * NSUB]
            sid = f"{rep}_{sp}"
            psA = pspool.tile([128, NSUB], f32, tag="psA", bufs=2,
                              name=f"psA_{sid}")
            psB = pspool.tile([128, NSUB], f32, tag="psB", bufs=2,
                              name=f"psB_{sid}")
            logit = pspool.tile([1, NSUB], f32, tag="logit", bufs=2,
                                name=f"logit_{sid}")
            psD = pspool.tile([128, NSUB], f32, tag="psD", name=f"psD_{sid}")
            psE = pspool.tile([128, NSUB], f32, tag="psE", name=f"psE_{sid}")
            accs = [None, None]   # DVE parities (hide RAW latency)
            atmps = []            # ACT-lin tmp tiles
            ndve = 0
            for t in range(NKT):
                if t in A_TILES:
                    nc.tensor.matmul(psA[:], w_sb[:, AOFF[t]:AOFF[t] + 128],
                                     rhs(t), start=(t == 0), stop=False)
                if t in B_TILES:
                    nc.tensor.matmul(psB[:], w_sb[:, BOFF[t]:BOFF[t] + 128],
                                     rhs(t), start=(t == 7), stop=(t == 20))
                if t == 20:
                    nc.tensor.matmul(psA[:], w_sb[:, S3OFF:S3OFF + 128],
                                     rhs(t), start=False, stop=True)
                    nc.tensor.matmul(psD[:], w_sb[:, DOFF:DOFF + 128],
                                     rhs(t), start=True, stop=True)
                    nc.tensor.matmul(psE[:], w_sb[:, EOFF:EOFF + 128],
                                     rhs(t), start=True, stop=True)
                # linear term: per-partition-scalar multiply-accumulate,
                # split across PE (M=1 matmuls grouped after the blocks),
                # ACT (scaled copies) and DVE (TSS+add pairs)
                wcol = lin_sb[:, t:t + 1]
                if t in PE_T:
                    pass  # emitted below, grouped with the reduce
                elif t in ACT_T:
                    at = spool.tile([128, NSUB], f16, tag=f"atmp{len(atmps)}",
                                    name=f"atmp{len(atmps)}_{sid}")
                    nc.scalar.activation(at[:], rhs(t), COPY, scale=wcol)
                    atmps.append(at)
                else:
                    par = ndve % 2
                    ndve += 1
                    if accs[par] is None:
                        at = spool.tile([128, NSUB], f16, tag=f"acc{par}",
                                        name=f"acc{par}_{sid}")
                        nc.vector.tensor_single_scalar(at[:], rhs(t),
                                                       wcol, MUL)
                        accs[par] = at
                    elif lin_stt:
                        nc.vector.scalar_tensor_tensor(
                            accs[par][:], rhs(t), wcol, accs[par][:],
                            MUL, ADD)
                    else:
                        tmp = spool.tile([128, NSUB], f16, tag=f"tmp{par}",
                                         name=f"tmp{par}_{sid}_{t}")
                        nc.vector.tensor_single_scalar(tmp[:], rhs(t),
                                                       wcol, MUL)
                        nc.vector.tensor_add(accs[par][:], accs[par][:],
                                             tmp[:])
            # linear-term M=1 matmuls, grouped so the PE switches from
            # 128-row to 1-row output tiles only once per sub
            for j, t in enumerate(PE_T):
                nc.tensor.matmul(logit[:], lin16_sb[:, t:t + 1],
                                 rhs(t), start=(j == 0), stop=False)
            # drains: PSUM -> fp16 SBUF on the ACT engine, in PE completion
            # order (B stops first, then A/D/E at t20)
            dB = spool.tile([128, NSUB], f16, tag="dB", name=f"dB_{sid}")
            nc.scalar.copy(dB[:], psB[:])
            dA = spool.tile([128, NSUB], f16, tag="dA", name=f"dA_{sid}")
            nc.scalar.copy(dA[:], psA[:])
            dD = spool.tile([128, NSUB], f16, tag="dD", name=f"dD_{sid}")
            nc.scalar.copy(dD[:], psD[:])
            dE = spool.tile([128, NSUB], f16, tag="dE", name=f"dE_{sid}")
            nc.scalar.copy(dE[:], psE[:])
            # retire older subs: reduce + sigmoid one sub behind (their
            # products were emitted at the head of this sub)
            depth = int(os.environ.get("FFM_DEPTH", "1"))
            while len(pend) >= depth:
                _finish(pend.pop(0))
            pend.append({"sid": sid, "dA": dA, "dB": dB, "dD": dD,
                         "dE": dE, "acc0": accs[0], "acc1": accs[1],
                         "atmps": atmps, "logit": logit, "col": s * NSUB})
        _products(pend[-1])
        for p in pend:
            _finish(p)

    if loop and repeat > 1:
        # benchmarking mode: run the identical body `repeat` times inside one
        # NEFF via a hardware loop, multiple passes per iteration so the
        # software pipeline flows across pass boundaries.
        if repeat % 16 == 0:
            with tc.For_i(0, repeat // 16, 1):
                _body(0, passes=16)
        elif repeat % 8 == 0:
            with tc.For_i(0, repeat // 8, 1):
                _body(0, passes=8)
        elif repeat % 4 == 0:
            with tc.For_i(0, repeat // 4, 1):
                _body(0, passes=4)
        elif repeat % 2 == 0:
            with tc.For_i(0, repeat // 2, 1):
                _body(0, passes=2)
        else:
            with tc.For_i(0, repeat, 1):
                _body(0)
    else:
        for rep in range(repeat):
            _body(rep)


_MODULES = {}


def get_module(repeat=1, loop=False):
    """Build (once per config) and return the compiled Bass module."""
    key = (repeat, loop)
    if key in _MODULES:
        return _MODULES[key]

    import concourse.bacc as bacc
    import concourse.tile as tile
    import concourse.mybir as mybir

    nc = bacc.Bacc("TRN2", debug=False, enable_asserts=False,
                   num_devices=NCORES)
    fvt_d = nc.dram_tensor("fvt", (NSUBS, 128, SW), mybir.dt.float16,
                           kind="ExternalInput").ap()
    w_d = nc.dram_tensor("wpack", (128, WF), mybir.dt.float16,
                         kind="ExternalInput").ap()
    lin_d = nc.dram_tensor("lin32", (128, NKT), mybir.dt.float32,
                           kind="ExternalInput").ap()
    lb_d = nc.dram_tensor("linb", (1, 1), mybir.dt.float32,
                          kind="ExternalInput").ap()
    ones_d = nc.dram_tensor("ones16", (128, 1), mybir.dt.float16,
                            kind="ExternalInput").ap()
    out_d = nc.dram_tensor("out", (1, BL), mybir.dt.float32,
                           kind="ExternalOutput").ap()

    with tile.TileContext(nc) as tc, ExitStack() as ctx:
        _trace_kernel(ctx, tc, out_d, fvt_d, w_d, lin_d, lb_d,
                      ones_d, repeat=repeat, loop=loop)
    nc.compile()
    _MODULES[key] = nc
    return nc


def prepare_in_maps(inputs):
    """Host-side sharding: batch-split fv, pack each shard sub-major as
    [NSUBS, 128, NKT*512] fp16 (one contiguous DMA per sub), replicate
    the packed weights."""
    fv = np.ascontiguousarray(np.asarray(inputs["feature_vector"], np.float32))
    assert fv.shape == (B, F)
    tables = {k: np.asarray(v, np.float32) for k, v in inputs.items()
              if k != "feature_vector"}
    w_pack = np.ascontiguousarray(_build_w_pack(tables), np.float16)
    lw = np.zeros(FP, np.float32)
    lw[:F] = tables["lin_w"][0]
    # -au*au correction rides the linear chain: a derived fv row holds
    # fv_2626^2 and its linear weight is -||A_u||^2
    lw[F + 1] = -float((tables["age_user_w"][0] ** 2).sum())
    lin32 = np.ascontiguousarray(lw.reshape(NKT, 128).T)
    lb = tables["lin_b"].reshape(1, 1)
    ones16 = np.ones((128, 1), np.float16)

    in_maps = []
    for c in range(NCORES):
        fvt = np.zeros((FP, BL), np.float16)
        fvt[:F] = fv[c * BL:(c + 1) * BL].T
        fvt[F + 1] = fv[c * BL:(c + 1) * BL, 2626] ** 2
        # [t*128+p, s*512+c] -> [s, p, t*512+c]
        fvt = np.ascontiguousarray(
            fvt.reshape(NKT, 128, NSUBS, NSUB).transpose(2, 1, 0, 3)
               .reshape(NSUBS, 128, SW))
        in_maps.append({"fvt": fvt, "wpack": w_pack, "lin32": lin32,
                        "linb": lb, "ones16": ones16})
    return in_maps


def kernel(**inputs) -> np.ndarray:
    # Tracing needs the axon NTFF hook, which this environment lacks; make
    # sure a stray BASS_TRACE=1 can't crash the run.
    os.environ["BASS_NEVER_TRACE"] = "1"
    from concourse import bass_utils

    in_maps = prepare_in_maps(inputs)
    nc = get_module()
    try:
        res = bass_utils.run_bass_kernel_spmd(nc, in_maps,
                                              core_ids=list(range(NCORES)))
    except Exception:
        # transient NRT device errors have been observed on this fabric;
        # one retry after a short pause usually succeeds
        import time
        time.sleep(15)
        res = bass_utils.run_bass_kernel_spmd(nc, in_maps,
                                              core_ids=list(range(NCORES)))
    out = np.concatenate([r["out"].reshape(BL) for r in res.results])
    return out.reshape(B, 1).astype(np.float32)


# revision 13
# speedup vs baseline: 1.0724x; 1.0311x over previous
"""Trainium2 Bass kernel for an FFM (field-aware factorization machine) forward pass.

Reference computation (all fp32):
    12 embedding matmuls over column slices of fv [32768, 2668], 15 pairwise
    dot-product cross terms, a linear layer and a sigmoid.

The kernel is DMA-bound: fv is 22 MB/core in fp16 (~66 us at effective HBM
bandwidth), so the design pushes every engine's busy time below the DMA
floor (measured per-op costs for a [128,512] tile: PE matmul 213ns, DVE
TSS 194 / TT 327 / STT ~750, Pool TT ~990 (Pool TSS/STT are unusable:
8us / unsupported), ACT copy ~500):

  * The 12 embeddings are packed as 64-row halves of 128-row weight blocks:
      A = [uu | ui+S3]   tiles 0..7 + an S3 matmul on tile 20
      B = [ti | tu+mu]   tiles 7..20
      D = [mi | au+gu],  E = [R | au+ou]   (tile 20 only)
    where R = au+gu+ou, S3 = ai+gi+oi. Since the cross terms contain
    (tu+mu)*(ui+S3), accumulating S3 into psA's high half merges two
    products into one. 25 block matmuls + 1 reduce matmul per sub.
  * Cross terms are 4 tensor_tensor half-products on drained fp16 tiles:
    uu*R, (ui+S3)*(tu+mu), mi*ti, (au+gu)*(au+ou); the -au*au correction
    rides the linear chain via a host-derived fv row holding fv_2626^2
    with linear weight -||A_u||^2. Products + fold adds run on the DVE,
    emitted at the HEAD of the next sub's program so the in-order DVE
    queue clears the retire path early and the single ones-matmul that
    closes the logit never stalls the PE (routing them through the Pool
    engine or the queue tail measured ~10 us slower).
  * The linear term is split 6/3/12: six K-tiles as M=1 fp16 PE matmuls
    into the logit PSUM (grouped after the blocks), three as ACT copies
    with a per-partition scale, the rest as DVE TSS-multiply + TT-add
    pairs (94+306 ns, cheaper than the fused STT's 752 ns on HW).
  * PSUM is drained to fp16 SBUF on the ACT engine; all DVE operands
    are then 2-byte + SBUF-only, enabling the DVE fast modes.
  * Sub n's reduce + sigmoid retire one sub late (software pipelining)
    so the in-order engine queues never stall on the previous sub's tail.
  * fv is streamed sub-major: one contiguous [128, 21*512] fp16 DMA per
    512-batch sub (host pre-arranges the layout), split across two HWDGE
    rings for bandwidth.

Distribution: data-parallel over the batch dim - each of the 8 cores gets
4096 rows, cast to fp16 host-side (halves HBM traffic; rel err ~4e-3 vs
the 2e-2 gate).
"""

import os
import numpy as np
from contextlib import ExitStack

B, F, D = 32768, 2668, 64
NCORES = 8
BL = B // NCORES          # batch rows per core
NKT = 21                  # feature K-tiles of 128
FP = NKT * 128            # padded feature dim (2688)
NSUB = 512                # matmul moving-dim (one fp32 PSUM bank)
NSUBS = BL // NSUB        # subs per core (8)
SW = NKT * NSUB           # packed width of one sub (10752)

# w_pack column layout: A t0..7 | B t7..20 | S3 | D | E  (128 cols per tile)
A_TILES = tuple(range(0, 8))
B_TILES = tuple(range(7, 21))
AOFF = {t: i * 128 for i, t in enumerate(A_TILES)}
BOFF = {t: (8 + i) * 128 for i, t in enumerate(B_TILES)}
S3OFF, DOFF, EOFF = 22 * 128, 23 * 128, 24 * 128
ZOFF = 25 * 128           # zero block (load-line probe dummies)
WF = 26 * 128

# linear-term K-tile split across engines (tunable)
PE_LIN = int(os.environ.get("FFM_PE_LIN", "6"))
ACT_LIN = int(os.environ.get("FFM_ACT_LIN", "3"))


def _lin_split():
    """Deterministic partition of the 21 K-tiles among PE/ACT/DVE."""
    order = list(range(NKT))
    pe_t = order[0::2][:PE_LIN]
    rest = [t for t in order if t not in pe_t]
    act_t = rest[0::2][:ACT_LIN]
    dve_t = [t for t in rest if t not in act_t]
    return tuple(pe_t), set(act_t), set(dve_t)


PE_T, ACT_T, DVE_T = _lin_split()


def _build_w_pack(inp):
    """Pack the block tables into one [128, WF] array laid out exactly as
    the SBUF weight tile wants it (partition k = row-in-K-tile)."""
    A_u, A_i = inp["age_user_w"], inp["age_item_w"]
    G_u, G_i = inp["gender_user_w"], inp["gender_item_w"]
    O_u, O_i = inp["occupation_user_w"], inp["occupation_item_w"]
    M_u, M_i = inp["movie_user_w"], inp["movie_item_w"]
    U_u, U_i = inp["userid_user_w"], inp["userid_item_w"]
    T_u, T_i = inp["itemid_user_w"], inp["itemid_item_w"]

    WA = np.zeros((FP, 128), np.float32)
    WA[0:943, 0:64] = U_u; WA[0:943, 64:128] = U_i
    WB = np.zeros((FP, 128), np.float32)
    WB[943:2625, 0:64] = T_i
    WB[943:2625, 64:128] = T_u; WB[2649:2668, 64:128] = M_u
    WS3 = np.zeros((FP, 128), np.float32)
    WS3[2626:2627, 64:128] += A_i; WS3[2626:2628, 64:128] += G_i
    WS3[2628:2649, 64:128] += O_i
    WD = np.zeros((FP, 128), np.float32)
    WD[2649:2668, 0:64] = M_i
    WD[2626:2627, 64:128] += A_u; WD[2626:2628, 64:128] += G_u
    WE = np.zeros((FP, 128), np.float32)
    WE[2626:2627, 0:64] += A_u; WE[2626:2628, 0:64] += G_u
    WE[2628:2649, 0:64] += O_u
    WE[2626:2627, 64:128] += A_u; WE[2628:2649, 64:128] += O_u

    w_pack = np.zeros((128, WF), np.float32)
    for t in A_TILES:
        w_pack[:, AOFF[t]:AOFF[t] + 128] = WA[t * 128:(t + 1) * 128]
    for t in B_TILES:
        w_pack[:, BOFF[t]:BOFF[t] + 128] = WB[t * 128:(t + 1) * 128]
    w_pack[:, S3OFF:S3OFF + 128] = WS3[20 * 128:21 * 128]
    w_pack[:, DOFF:DOFF + 128] = WD[20 * 128:21 * 128]
    w_pack[:, EOFF:EOFF + 128] = WE[20 * 128:21 * 128]
    return w_pack


def _trace_kernel(ctx: ExitStack, tc, out_d, fvt_d, w_d, lin_d, lb_d,
                  ones_d, repeat=1, loop=False):
    import concourse.mybir as mybir

    nc = tc.nc
    f32 = mybir.dt.float32
    f16 = mybir.dt.float16
    MUL = mybir.AluOpType.mult
    ADD = mybir.AluOpType.add
    COPY = mybir.ActivationFunctionType.Copy

    wpool = ctx.enter_context(tc.tile_pool(name="wpool", bufs=1))
    w_sb = wpool.tile([128, WF], f16, name="w_sb")
    nc.sync.dma_start(w_sb[:, 0:128], w_d[:, 0:128])
    nc.sync.dma_start(w_sb[:, 128:WF], w_d[:, 128:WF])
    lin_sb = wpool.tile([128, NKT], f32, name="lin_sb")
    nc.sync.dma_start(lin_sb[:], lin_d[:])
    lin16_sb = wpool.tile([128, NKT], f16, name="lin16_sb")
    nc.gpsimd.dma_start(lin16_sb[:], lin_d[:])  # casting DMA (f32 -> f16)
    lb_sb = wpool.tile([1, 1], f32, name="lb_sb")
    nc.sync.dma_start(lb_sb[:], lb_d[:])
    ones_sb = wpool.tile([128, 1], f16, name="ones_sb")
    nc.sync.dma_start(ones_sb[:], ones_d[:])

    fpool = ctx.enter_context(tc.tile_pool(
        name="fpool", bufs=int(os.environ.get("FFM_FBUFS", "4"))))
    pspool = ctx.enter_context(tc.tile_pool(name="pspool", bufs=1, space="PSUM"))
    spool = ctx.enter_context(tc.tile_pool(name="spool", bufs=3))
    opool = ctx.enter_context(tc.tile_pool(name="opool", bufs=2))
    out_eng = {"sync": nc.sync, "scalar": nc.scalar,
               "gpsimd": nc.gpsimd}[os.environ.get("FFM_OUTDMA", "sync")]
    lin_stt = os.environ.get("FFM_LIN_STT", "0") == "1"
    pool_prod = int(os.environ.get("FFM_POOL_PROD", "0"))
    pool_folds = int(os.environ.get("FFM_POOL_FOLDS", "0"))
    reduces = int(os.environ.get("FFM_REDUCES", "1"))
    # load-line probes: extra no-op work to measure an engine's marginal cost
    xpe = int(os.environ.get("FFM_XPE", "0"))
    xdve = int(os.environ.get("FFM_XDVE", "0"))

    HALF = SW // 2  # ring-split point of the packed sub row

    def _products(p):
        """Cross products + folds for a sub whose drains landed a sub ago."""
        sid = p["sid"]
        dA, dB, dD, dE = p["dA"], p["dB"], p["dD"], p["dE"]
        st1 = spool.tile([128, NSUB], f16, tag="st1", name=f"st1_{sid}")
        st2 = spool.tile([128, NSUB], f16, tag="st2", name=f"st2_{sid}")
        prods = [
            (st1, slice(0, 64), dA, dE),        # uu*R
            (st1, slice(64, 128), dA, dB),      # (ui+S3)*(tu+mu)
            (st2, slice(0, 64), dD, dB),        # mi*ti
            (st2, slice(64, 128), dD, dE),      # (au+gu)*(au+ou) [-au^2 in lin]
        ]
        for j, (dst, sl, x, y) in enumerate(prods):
            eng = nc.gpsimd if j < pool_prod else nc.vector
            eng.tensor_mul(dst[sl], x[sl], y[sl])
        # fold ACT-lin tmp tiles into the DVE accumulators
        accs = [a for a in (p["acc0"], p["acc1"]) if a is not None]
        srcs = [st1, st2] + accs
        for i, tmp in enumerate(p["atmps"]):
            if accs:
                tgt = accs[i % len(accs)]
                nc.vector.tensor_add(tgt[:], tgt[:], tmp[:])
            else:
                srcs.append(tmp)
        # fold source pairs so fewer PE reduce matmuls are needed
        nf = 0
        while len(srcs) > max(reduces, 1):
            a = srcs.pop(0); b = srcs.pop(0)
            eng = nc.gpsimd if nf < pool_folds else nc.vector
            eng.tensor_add(a[:], a[:], b[:])
            srcs.append(a)
            nf += 1
        p["red"] = tuple(srcs)

    def _finish(p):
        """Reduce + sigmoid + store for a sub whose products are complete."""
        red, col, logit = p["red"], p["col"], p["logit"]
        for j, srct in enumerate(red):
            nc.tensor.matmul(logit[:], ones_sb[:], srct[:],
                             start=(j == 0 and not PE_T),
                             stop=(j == len(red) - 1))
        out_sb = opool.tile([1, NSUB], f32, tag="out", name=f"out_{col}")
        nc.scalar.activation(out_sb[:], logit[:],
                             mybir.ActivationFunctionType.Sigmoid,
                             bias=lb_sb[0:1, 0:1], scale=1.0)
        out_eng.dma_start(out_d[0:1, col:col + NSUB], out_sb[:])

    def _body(rep, passes=1):
        pend = []  # software-pipeline: sub n retires during sub n+1
        for sp in range(passes * NSUBS):
            s = sp % NSUBS
            big = fpool.tile([128, SW], f16, tag="fv", name=f"fv_{rep}_{sp}")
            nc.sync.dma_start(big[:, 0:HALF], fvt_d[s][:, 0:HALF])
            nc.scalar.dma_start(big[:, HALF:SW], fvt_d[s][:, HALF:SW])
            # retire path of the previous sub FIRST: its products/folds land
            # at the head of the in-order DVE queue (its drains are already
            # done), so the PE reduce emitted at this sub's tail never stalls
            if pend:
                _products(pend[-1])

            def rhs(t):
                return big[:, t * NSUB:(t + 1) * NSUB]
            sid = f"{rep}_{sp}"
            psA = pspool.tile([128, NSUB], f32, tag="psA", bufs=2,
                              name=f"psA_{sid}")
            psB = pspool.tile([128, NSUB], f32, tag="psB", bufs=2,
                              name=f"psB_{sid}")
            logit = pspool.tile([1, NSUB], f32, tag="logit", bufs=2,
                                name=f"logit_{sid}")
            psD = pspool.tile([128, NSUB], f32, tag="psD", name=f"psD_{sid}")
            psE = pspool.tile([128, NSUB], f32, tag="psE", name=f"psE_{sid}")
            accs = [None, None]   # DVE parities (hide RAW latency)
            atmps = []            # ACT-lin tmp tiles
            ndve = 0
            for t in range(NKT):
                if t in A_TILES:
                    nc.tensor.matmul(psA[:], w_sb[:, AOFF[t]:AOFF[t] + 128],
                                     rhs(t), start=(t == 0), stop=False)
                if t in B_TILES:
                    nc.tensor.matmul(psB[:], w_sb[:, BOFF[t]:BOFF[t] + 128],
                                     rhs(t), start=(t == 7), stop=(t == 20))
                if t == 10:
                    for _x in range(xpe):   # zero-weight: psB unchanged
                        nc.tensor.matmul(psB[:], w_sb[:, ZOFF:ZOFF + 128],
                                         rhs(t), start=False, stop=False)
                    if xdve:
                        xsc = spool.tile([128, NSUB], f16, tag="xdve",
                                         name=f"xdve_{sid}")
                        for _x in range(xdve):
                            nc.vector.tensor_mul(xsc[:], rhs(0), rhs(1))
                if t == 20:
                    nc.tensor.matmul(psA[:], w_sb[:, S3OFF:S3OFF + 128],
                                     rhs(t), start=False, stop=True)
                    nc.tensor.matmul(psD[:], w_sb[:, DOFF:DOFF + 128],
                                     rhs(t), start=True, stop=True)
                    nc.tensor.matmul(psE[:], w_sb[:, EOFF:EOFF + 128],
                                     rhs(t), start=True, stop=True)
                # linear term: per-partition-scalar multiply-accumulate,
                # split across PE (M=1 matmuls grouped after the blocks),
                # ACT (scaled copies) and DVE (TSS+add pairs)
                wcol = lin_sb[:, t:t + 1]
                if t in PE_T:
                    pass  # emitted below, grouped with the reduce
                elif t in ACT_T:
                    at = spool.tile([128, NSUB], f16, tag=f"atmp{len(atmps)}",
                                    name=f"atmp{len(atmps)}_{sid}")
                    nc.scalar.activation(at[:], rhs(t), COPY, scale=wcol)
                    atmps.append(at)
                else:
                    par = ndve % 2
                    ndve += 1
                    if accs[par] is None:
                        at = spool.tile([128, NSUB], f16, tag=f"acc{par}",
                                        name=f"acc{par}_{sid}")
                        nc.vector.tensor_single_scalar(at[:], rhs(t),
                                                       wcol, MUL)
                        accs[par] = at
                    elif lin_stt:
                        nc.vector.scalar_tensor_tensor(
                            accs[par][:], rhs(t), wcol, accs[par][:],
                            MUL, ADD)
                    else:
                        tmp = spool.tile([128, NSUB], f16, tag=f"tmp{par}",
                                         name=f"tmp{par}_{sid}_{t}")
                        nc.vector.tensor_single_scalar(tmp[:], rhs(t),
                                                       wcol, MUL)
                        nc.vector.tensor_add(accs[par][:], accs[par][:],
                                             tmp[:])
            # linear-term M=1 matmuls, grouped so the PE switches from
            # 128-row to 1-row output tiles only once per sub
            for j, t in enumerate(PE_T):
                nc.tensor.matmul(logit[:], lin16_sb[:, t:t + 1],
                                 rhs(t), start=(j == 0), stop=False)
            # drains: PSUM -> fp16 SBUF on the ACT engine, in PE completion
            # order (B stops first, then A/D/E at t20)
            dB = spool.tile([128, NSUB], f16, tag="dB", name=f"dB_{sid}")
            nc.scalar.copy(dB[:], psB[:])
            dA = spool.tile([128, NSUB], f16, tag="dA", name=f"dA_{sid}")
            nc.scalar.copy(dA[:], psA[:])
            dD = spool.tile([128, NSUB], f16, tag="dD", name=f"dD_{sid}")
            nc.scalar.copy(dD[:], psD[:])
            dE = spool.tile([128, NSUB], f16, tag="dE", name=f"dE_{sid}")
            nc.scalar.copy(dE[:], psE[:])
            # retire older subs: reduce + sigmoid one sub behind (their
            # products were emitted at the head of this sub)
            depth = int(os.environ.get("FFM_DEPTH", "1"))
            while len(pend) >= depth:
                _finish(pend.pop(0))
            pend.append({"sid": sid, "dA": dA, "dB": dB, "dD": dD,
                         "dE": dE, "acc0": accs[0], "acc1": accs[1],
                         "atmps": atmps, "logit": logit, "col": s * NSUB})
        _products(pend[-1])
        for p in pend:
            _finish(p)

    if loop and repeat > 1:
        # benchmarking mode: run the identical body `repeat` times inside one
        # NEFF via a hardware loop, multiple passes per iteration so the
        # software pipeline flows across pass boundaries.
        if repeat % 16 == 0:
            with tc.For_i(0, repeat // 16, 1):
                _body(0, passes=16)
        elif repeat % 8 == 0:
            with tc.For_i(0, repeat // 8, 1):
                _body(0, passes=8)
        elif repeat % 4 == 0:
            with tc.For_i(0, repeat // 4, 1):
                _body(0, passes=4)
        elif repeat % 2 == 0:
            with tc.For_i(0, repeat // 2, 1):
                _body(0, passes=2)
        else:
            with tc.For_i(0, repeat, 1):
                _body(0)
    else:
        for rep in range(repeat):
            _body(rep)


_MODULES = {}


def get_module(repeat=1, loop=False):
    """Build (once per config) and return the compiled Bass module."""
    key = (repeat, loop)
    if key in _MODULES:
        return _MODULES[key]

    import concourse.bacc as bacc
    import concourse.tile as tile
    import concourse.mybir as mybir

    nc = bacc.Bacc("TRN2", debug=False, enable_asserts=False,
                   num_devices=NCORES)
    fvt_d = nc.dram_tensor("fvt", (NSUBS, 128, SW), mybir.dt.float16,
                           kind="ExternalInput").ap()
    w_d = nc.dram_tensor("wpack", (128, WF), mybir.dt.float16,
                         kind="ExternalInput").ap()
    lin_d = nc.dram_tensor("lin32", (128, NKT), mybir.dt.float32,
                           kind="ExternalInput").ap()
    lb_d = nc.dram_tensor("linb", (1, 1), mybir.dt.float32,
                          kind="ExternalInput").ap()
    ones_d = nc.dram_tensor("ones16", (128, 1), mybir.dt.float16,
                            kind="ExternalInput").ap()
    out_d = nc.dram_tensor("out", (1, BL), mybir.dt.float32,
                           kind="ExternalOutput").ap()

    with tile.TileContext(nc) as tc, ExitStack() as ctx:
        _trace_kernel(ctx, tc, out_d, fvt_d, w_d, lin_d, lb_d,
                      ones_d, repeat=repeat, loop=loop)
    nc.compile()
    _MODULES[key] = nc
    return nc


def prepare_in_maps(inputs):
    """Host-side sharding: batch-split fv, pack each shard sub-major as
    [NSUBS, 128, NKT*512] fp16 (one contiguous DMA per sub), replicate
    the packed weights."""
    fv = np.ascontiguousarray(np.asarray(inputs["feature_vector"], np.float32))
    assert fv.shape == (B, F)
    tables = {k: np.asarray(v, np.float32) for k, v in inputs.items()
              if k != "feature_vector"}
    w_pack = np.ascontiguousarray(_build_w_pack(tables), np.float16)
    lw = np.zeros(FP, np.float32)
    lw[:F] = tables["lin_w"][0]
    # -au*au correction rides the linear chain: a derived fv row holds
    # fv_2626^2 and its linear weight is -||A_u||^2
    lw[F + 1] = -float((tables["age_user_w"][0] ** 2).sum())
    lin32 = np.ascontiguousarray(lw.reshape(NKT, 128).T)
    lb = tables["lin_b"].reshape(1, 1)
    ones16 = np.ones((128, 1), np.float16)

    in_maps = []
    for c in range(NCORES):
        fvt = np.zeros((FP, BL), np.float16)
        fvt[:F] = fv[c * BL:(c + 1) * BL].T
        fvt[F + 1] = fv[c * BL:(c + 1) * BL, 2626] ** 2
        # [t*128+p, s*512+c] -> [s, p, t*512+c]
        fvt = np.ascontiguousarray(
            fvt.reshape(NKT, 128, NSUBS, NSUB).transpose(2, 1, 0, 3)
               .reshape(NSUBS, 128, SW))
        in_maps.append({"fvt": fvt, "wpack": w_pack, "lin32": lin32,
                        "linb": lb, "ones16": ones16})
    return in_maps


def kernel(**inputs) -> np.ndarray:
    # Tracing needs the axon NTFF hook, which this environment lacks; make
    # sure a stray BASS_TRACE=1 can't crash the run.
    os.environ["BASS_NEVER_TRACE"] = "1"
    from concourse import bass_utils

    in_maps = prepare_in_maps(inputs)
    nc = get_module()
    try:
        res = bass_utils.run_bass_kernel_spmd(nc, in_maps,
                                              core_ids=list(range(NCORES)))
    except Exception:
        # transient NRT device errors have been observed on this fabric;
        # one retry after a short pause usually succeeds
        import time
        time.sleep(15)
        res = bass_utils.run_bass_kernel_spmd(nc, in_maps,
                                              core_ids=list(range(NCORES)))
    out = np.concatenate([r["out"].reshape(BL) for r in res.results])
    return out.reshape(B, 1).astype(np.float32)


# revision 16
# speedup vs baseline: 1.1747x; 1.0954x over previous
"""Trainium2 Bass kernel for an FFM (field-aware factorization machine) forward pass.

Reference computation (all fp32):
    12 embedding matmuls over column slices of fv [32768, 2668], 15 pairwise
    dot-product cross terms, a linear layer and a sigmoid.

The kernel is DMA-bound: fv is 22 MB/core in fp16 (~66 us at effective HBM
bandwidth), so the design pushes every engine's busy time below the DMA
floor (measured per-op costs for a [128,512] tile: PE matmul 213ns, DVE
TSS 194 / TT 327 / STT ~750, Pool TT ~990 (Pool TSS/STT are unusable:
8us / unsupported), ACT copy ~500):

  * The 12 embeddings are packed as 64-row halves of 128-row weight blocks:
      A = [uu | ui+S3]   tiles 0..7 + an S3 matmul on tile 20
      B = [ti | tu+mu]   tiles 7..20
      D = [mi | au+gu],  E = [R | au+ou]   (tile 20 only)
    where R = au+gu+ou, S3 = ai+gi+oi. Since the cross terms contain
    (tu+mu)*(ui+S3), accumulating S3 into psA's high half merges two
    products into one. 25 block matmuls + 1 reduce matmul per sub.
  * Cross terms are 4 tensor_tensor half-products on drained fp16 tiles:
    uu*R, (ui+S3)*(tu+mu), mi*ti, (au+gu)*(au+ou); the -au*au correction
    rides the linear chain via a host-derived fv row holding fv_2626^2
    with linear weight -||A_u||^2. Products + fold adds run on the DVE,
    emitted at the HEAD of the next sub's program so the in-order DVE
    queue clears the retire path early and the single ones-matmul that
    closes the logit never stalls the PE (routing them through the Pool
    engine or the queue tail measured ~10 us slower).
  * The linear term is split 6/3/12: six K-tiles as M=1 fp16 PE matmuls
    into the logit PSUM (grouped after the blocks), three as ACT copies
    with a per-partition scale, the rest as DVE TSS-multiply + TT-add
    pairs (94+306 ns, cheaper than the fused STT's 752 ns on HW).
  * PSUM is drained to fp16 SBUF on the ACT engine; all DVE operands
    are then 2-byte + SBUF-only, enabling the DVE fast modes.
  * Sub n's reduce + sigmoid retire one sub late (software pipelining)
    so the in-order engine queues never stall on the previous sub's tail.
  * fv is streamed sub-major: one contiguous [128, 21*512] fp16 DMA per
    512-batch sub (host pre-arranges the layout), split across two HWDGE
    rings for bandwidth.

Distribution: data-parallel over the batch dim - each of the 8 cores gets
4096 rows, cast to fp16 host-side (halves HBM traffic; rel err ~4e-3 vs
the 2e-2 gate).
"""

import os
import numpy as np
from contextlib import ExitStack

B, F, D = 32768, 2668, 64
NCORES = 8
BL = B // NCORES          # batch rows per core
NKT = 21                  # feature K-tiles of 128
FP = NKT * 128            # padded feature dim (2688)
NSUB = 512                # matmul moving-dim (one fp32 PSUM bank)
NSUBS = BL // NSUB        # subs per core (8)
SW = NKT * NSUB           # packed width of one sub (10752)

# w_pack column layout: A t0..7 | B t7..20 | S3 | D | E  (128 cols per tile)
A_TILES = tuple(range(0, 8))
B_TILES = tuple(range(7, 21))
AOFF = {t: i * 128 for i, t in enumerate(A_TILES)}
BOFF = {t: (8 + i) * 128 for i, t in enumerate(B_TILES)}
S3OFF, DOFF, EOFF = 22 * 128, 23 * 128, 24 * 128
ZOFF = 25 * 128           # zero block (load-line probe dummies)
WF = 26 * 128

# linear-term K-tile split across engines (tunable)
PE_LIN = int(os.environ.get("FFM_PE_LIN", "6"))
ACT_LIN = int(os.environ.get("FFM_ACT_LIN", "3"))


def _lin_split():
    """Deterministic partition of the 21 K-tiles among PE/ACT/DVE."""
    order = list(range(NKT))
    pe_t = order[0::2][:PE_LIN]
    rest = [t for t in order if t not in pe_t]
    act_t = rest[0::2][:ACT_LIN]
    dve_t = [t for t in rest if t not in act_t]
    return tuple(pe_t), set(act_t), set(dve_t)


PE_T, ACT_T, DVE_T = _lin_split()


def _build_w_pack(inp):
    """Pack the block tables into one [128, WF] array laid out exactly as
    the SBUF weight tile wants it (partition k = row-in-K-tile)."""
    A_u, A_i = inp["age_user_w"], inp["age_item_w"]
    G_u, G_i = inp["gender_user_w"], inp["gender_item_w"]
    O_u, O_i = inp["occupation_user_w"], inp["occupation_item_w"]
    M_u, M_i = inp["movie_user_w"], inp["movie_item_w"]
    U_u, U_i = inp["userid_user_w"], inp["userid_item_w"]
    T_u, T_i = inp["itemid_user_w"], inp["itemid_item_w"]

    WA = np.zeros((FP, 128), np.float32)
    WA[0:943, 0:64] = U_u; WA[0:943, 64:128] = U_i
    WB = np.zeros((FP, 128), np.float32)
    WB[943:2625, 0:64] = T_i
    WB[943:2625, 64:128] = T_u; WB[2649:2668, 64:128] = M_u
    WS3 = np.zeros((FP, 128), np.float32)
    WS3[2626:2627, 64:128] += A_i; WS3[2626:2628, 64:128] += G_i
    WS3[2628:2649, 64:128] += O_i
    WD = np.zeros((FP, 128), np.float32)
    WD[2649:2668, 0:64] = M_i
    WD[2626:2627, 64:128] += A_u; WD[2626:2628, 64:128] += G_u
    WE = np.zeros((FP, 128), np.float32)
    WE[2626:2627, 0:64] += A_u; WE[2626:2628, 0:64] += G_u
    WE[2628:2649, 0:64] += O_u
    WE[2626:2627, 64:128] += A_u; WE[2628:2649, 64:128] += O_u

    w_pack = np.zeros((128, WF), np.float32)
    for t in A_TILES:
        w_pack[:, AOFF[t]:AOFF[t] + 128] = WA[t * 128:(t + 1) * 128]
    for t in B_TILES:
        w_pack[:, BOFF[t]:BOFF[t] + 128] = WB[t * 128:(t + 1) * 128]
    w_pack[:, S3OFF:S3OFF + 128] = WS3[20 * 128:21 * 128]
    w_pack[:, DOFF:DOFF + 128] = WD[20 * 128:21 * 128]
    w_pack[:, EOFF:EOFF + 128] = WE[20 * 128:21 * 128]
    return w_pack


def _trace_kernel(ctx: ExitStack, tc, out_d, fvt_d, w_d, lin_d, lb_d,
                  ones_d, repeat=1, loop=False):
    import concourse.mybir as mybir

    nc = tc.nc
    f32 = mybir.dt.float32
    f16 = mybir.dt.float16
    MUL = mybir.AluOpType.mult
    ADD = mybir.AluOpType.add
    COPY = mybir.ActivationFunctionType.Copy

    wpool = ctx.enter_context(tc.tile_pool(name="wpool", bufs=1))
    w_sb = wpool.tile([128, WF], f16, name="w_sb")
    nc.sync.dma_start(w_sb[:, 0:128], w_d[:, 0:128])
    nc.sync.dma_start(w_sb[:, 128:WF], w_d[:, 128:WF])
    lin_sb = wpool.tile([128, NKT], f32, name="lin_sb")
    nc.sync.dma_start(lin_sb[:], lin_d[:])
    lin16_sb = wpool.tile([128, NKT], f16, name="lin16_sb")
    nc.gpsimd.dma_start(lin16_sb[:], lin_d[:])  # casting DMA (f32 -> f16)
    lb_sb = wpool.tile([1, 1], f32, name="lb_sb")
    nc.sync.dma_start(lb_sb[:], lb_d[:])
    ones_sb = wpool.tile([128, 1], f16, name="ones_sb")
    nc.sync.dma_start(ones_sb[:], ones_d[:])

    fpool = ctx.enter_context(tc.tile_pool(
        name="fpool", bufs=int(os.environ.get("FFM_FBUFS", "4"))))
    pspool = ctx.enter_context(tc.tile_pool(name="pspool", bufs=1, space="PSUM"))
    spool = ctx.enter_context(tc.tile_pool(name="spool", bufs=3))
    opool = ctx.enter_context(tc.tile_pool(name="opool", bufs=2))
    out_eng = {"sync": nc.sync, "scalar": nc.scalar,
               "gpsimd": nc.gpsimd}[os.environ.get("FFM_OUTDMA", "sync")]
    lin_stt = os.environ.get("FFM_LIN_STT", "0") == "1"
    pool_prod = int(os.environ.get("FFM_POOL_PROD", "0"))
    pool_folds = int(os.environ.get("FFM_POOL_FOLDS", "0"))
    reduces = int(os.environ.get("FFM_REDUCES", "1"))
    # load-line probes: extra no-op work to measure an engine's marginal cost
    xpe = int(os.environ.get("FFM_XPE", "0"))
    xdve = int(os.environ.get("FFM_XDVE", "0"))
    # software-pipeline depth: subs between drain and reduce/sigmoid retire.
    # PSUM budget (8 banks) forces psB single-buffered at depth 2:
    # psA 2 + psB (2|1) + psD 1 + psE 1 + logit (depth+1) = 8 either way.
    depth = int(os.environ.get("FFM_DEPTH", "1"))
    psb_bufs = 2 if depth == 1 else 1
    logit_bufs = depth + 1

    HALF = SW // 2  # ring-split point of the packed sub row

    def _products(p):
        """Cross products + folds for a sub whose drains landed a sub ago."""
        sid = p["sid"]
        dA, dB, dD, dE = p["dA"], p["dB"], p["dD"], p["dE"]
        st1 = spool.tile([128, NSUB], f16, tag="st1", name=f"st1_{sid}")
        st2 = spool.tile([128, NSUB], f16, tag="st2", name=f"st2_{sid}")
        prods = [
            (st1, slice(0, 64), dA, dE),        # uu*R
            (st1, slice(64, 128), dA, dB),      # (ui+S3)*(tu+mu)
            (st2, slice(0, 64), dD, dB),        # mi*ti
            (st2, slice(64, 128), dD, dE),      # (au+gu)*(au+ou) [-au^2 in lin]
        ]
        for j, (dst, sl, x, y) in enumerate(prods):
            eng = nc.gpsimd if j < pool_prod else nc.vector
            eng.tensor_mul(dst[sl], x[sl], y[sl])
        # fold ACT-lin tmp tiles into the DVE accumulators
        accs = [a for a in (p["acc0"], p["acc1"]) if a is not None]
        srcs = [st1, st2] + accs
        for i, tmp in enumerate(p["atmps"]):
            if accs:
                tgt = accs[i % len(accs)]
                nc.vector.tensor_add(tgt[:], tgt[:], tmp[:])
            else:
                srcs.append(tmp)
        # fold source pairs so fewer PE reduce matmuls are needed
        nf = 0
        while len(srcs) > max(reduces, 1):
            a = srcs.pop(0); b = srcs.pop(0)
            eng = nc.gpsimd if nf < pool_folds else nc.vector
            eng.tensor_add(a[:], a[:], b[:])
            srcs.append(a)
            nf += 1
        p["red"] = tuple(srcs)

    def _finish(p):
        """Reduce + sigmoid + store for a sub whose products are complete."""
        red, col, logit = p["red"], p["col"], p["logit"]
        for j, srct in enumerate(red):
            nc.tensor.matmul(logit[:], ones_sb[:], srct[:],
                             start=(j == 0 and not PE_T),
                             stop=(j == len(red) - 1))
        out_sb = opool.tile([1, NSUB], f32, tag="out", name=f"out_{col}")
        nc.scalar.activation(out_sb[:], logit[:],
                             mybir.ActivationFunctionType.Sigmoid,
                             bias=lb_sb[0:1, 0:1], scale=1.0)
        out_eng.dma_start(out_d[0:1, col:col + NSUB], out_sb[:])

    def _body(rep, passes=1):
        pend = []  # software-pipeline: sub n retires during sub n+1
        for sp in range(passes * NSUBS):
            s = sp % NSUBS
            big = fpool.tile([128, SW], f16, tag="fv", name=f"fv_{rep}_{sp}")
            nc.sync.dma_start(big[:, 0:HALF], fvt_d[s][:, 0:HALF])
            nc.scalar.dma_start(big[:, HALF:SW], fvt_d[s][:, HALF:SW])
            # retire path of the previous sub FIRST: its products/folds land
            # at the head of the in-order DVE queue (its drains are already
            # done), so the PE reduce emitted at this sub's tail never stalls
            if pend:
                _products(pend[-1])

            def rhs(t):
                return big[:, t * NSUB:(t + 1) * NSUB]
            sid = f"{rep}_{sp}"
            psA = pspool.tile([128, NSUB], f32, tag="psA", bufs=2,
                              name=f"psA_{sid}")
            psB = pspool.tile([128, NSUB], f32, tag="psB", bufs=psb_bufs,
                              name=f"psB_{sid}")
            logit = pspool.tile([1, NSUB], f32, tag="logit", bufs=logit_bufs,
                                name=f"logit_{sid}")
            psD = pspool.tile([128, NSUB], f32, tag="psD", name=f"psD_{sid}")
            psE = pspool.tile([128, NSUB], f32, tag="psE", name=f"psE_{sid}")
            accs = [None, None]   # DVE parities (hide RAW latency)
            atmps = []            # ACT-lin tmp tiles
            ndve = 0
            for t in range(NKT):
                if t in A_TILES:
                    nc.tensor.matmul(psA[:], w_sb[:, AOFF[t]:AOFF[t] + 128],
                                     rhs(t), start=(t == 0), stop=False)
                if t in B_TILES:
                    nc.tensor.matmul(psB[:], w_sb[:, BOFF[t]:BOFF[t] + 128],
                                     rhs(t), start=(t == 7), stop=(t == 20))
                if t == 10:
                    for _x in range(xpe):   # zero-weight: psB unchanged
                        nc.tensor.matmul(psB[:], w_sb[:, ZOFF:ZOFF + 128],
                                         rhs(t), start=False, stop=False)
                    if xdve:
                        xsc = spool.tile([128, NSUB], f16, tag="xdve",
                                         name=f"xdve_{sid}")
                        for _x in range(xdve):
                            nc.vector.tensor_mul(xsc[:], rhs(0), rhs(1))
                if t == 20:
                    nc.tensor.matmul(psA[:], w_sb[:, S3OFF:S3OFF + 128],
                                     rhs(t), start=False, stop=True)
                    nc.tensor.matmul(psD[:], w_sb[:, DOFF:DOFF + 128],
                                     rhs(t), start=True, stop=True)
                    nc.tensor.matmul(psE[:], w_sb[:, EOFF:EOFF + 128],
                                     rhs(t), start=True, stop=True)
                # linear term: per-partition-scalar multiply-accumulate,
                # split across PE (M=1 matmuls grouped after the blocks),
                # ACT (scaled copies) and DVE (TSS+add pairs)
                wcol = lin_sb[:, t:t + 1]
                if t in PE_T:
                    pass  # emitted below, grouped with the reduce
                elif t in ACT_T:
                    at = spool.tile([128, NSUB], f16, tag=f"atmp{len(atmps)}",
                                    name=f"atmp{len(atmps)}_{sid}")
                    nc.scalar.activation(at[:], rhs(t), COPY, scale=wcol)
                    atmps.append(at)
                else:
                    par = ndve % 2
                    ndve += 1
                    if accs[par] is None:
                        at = spool.tile([128, NSUB], f16, tag=f"acc{par}",
                                        name=f"acc{par}_{sid}")
                        nc.vector.tensor_single_scalar(at[:], rhs(t),
                                                       wcol, MUL)
                        accs[par] = at
                    elif lin_stt:
                        nc.vector.scalar_tensor_tensor(
                            accs[par][:], rhs(t), wcol, accs[par][:],
                            MUL, ADD)
                    else:
                        tmp = spool.tile([128, NSUB], f16, tag=f"tmp{par}",
                                         name=f"tmp{par}_{sid}_{t}")
                        nc.vector.tensor_single_scalar(tmp[:], rhs(t),
                                                       wcol, MUL)
                        nc.vector.tensor_add(accs[par][:], accs[par][:],
                                             tmp[:])
            # linear-term M=1 matmuls, grouped so the PE switches from
            # 128-row to 1-row output tiles only once per sub
            for j, t in enumerate(PE_T):
                nc.tensor.matmul(logit[:], lin16_sb[:, t:t + 1],
                                 rhs(t), start=(j == 0), stop=False)
            # drains: PSUM -> fp16 SBUF on the ACT engine, in PE completion
            # order (B stops first, then A/D/E at t20)
            dB = spool.tile([128, NSUB], f16, tag="dB", name=f"dB_{sid}")
            nc.scalar.copy(dB[:], psB[:])
            dA = spool.tile([128, NSUB], f16, tag="dA", name=f"dA_{sid}")
            nc.scalar.copy(dA[:], psA[:])
            dD = spool.tile([128, NSUB], f16, tag="dD", name=f"dD_{sid}")
            nc.scalar.copy(dD[:], psD[:])
            dE = spool.tile([128, NSUB], f16, tag="dE", name=f"dE_{sid}")
            nc.scalar.copy(dE[:], psE[:])
            # retire older subs: reduce + sigmoid `depth` subs behind (their
            # products were emitted at the head of the following sub)
            while len(pend) >= depth:
                _finish(pend.pop(0))
            pend.append({"sid": sid, "dA": dA, "dB": dB, "dD": dD,
                         "dE": dE, "acc0": accs[0], "acc1": accs[1],
                         "atmps": atmps, "logit": logit, "col": s * NSUB})
        _products(pend[-1])
        for p in pend:
            _finish(p)

    if loop and repeat > 1:
        # benchmarking mode: run the identical body `repeat` times inside one
        # NEFF via a hardware loop, multiple passes per iteration so the
        # software pipeline flows across pass boundaries.
        if repeat % 16 == 0:
            with tc.For_i(0, repeat // 16, 1):
                _body(0, passes=16)
        elif repeat % 8 == 0:
            with tc.For_i(0, repeat // 8, 1):
                _body(0, passes=8)
        elif repeat % 4 == 0:
            with tc.For_i(0, repeat // 4, 1):
                _body(0, passes=4)
        elif repeat % 2 == 0:
            with tc.For_i(0, repeat // 2, 1):
                _body(0, passes=2)
        else:
            with tc.For_i(0, repeat, 1):
                _body(0)
    else:
        for rep in range(repeat):
            _body(rep)


_MODULES = {}


def get_module(repeat=1, loop=False):
    """Build (once per config) and return the compiled Bass module."""
    key = (repeat, loop)
    if key in _MODULES:
        return _MODULES[key]

    import concourse.bacc as bacc
    import concourse.tile as tile
    import concourse.mybir as mybir

    nc = bacc.Bacc("TRN2", debug=False, enable_asserts=False,
                   num_devices=NCORES)
    fvt_d = nc.dram_tensor("fvt", (NSUBS, 128, SW), mybir.dt.float16,
                           kind="ExternalInput").ap()
    w_d = nc.dram_tensor("wpack", (128, WF), mybir.dt.float16,
                         kind="ExternalInput").ap()
    lin_d = nc.dram_tensor("lin32", (128, NKT), mybir.dt.float32,
                           kind="ExternalInput").ap()
    lb_d = nc.dram_tensor("linb", (1, 1), mybir.dt.float32,
                          kind="ExternalInput").ap()
    ones_d = nc.dram_tensor("ones16", (128, 1), mybir.dt.float16,
                            kind="ExternalInput").ap()
    out_d = nc.dram_tensor("out", (1, BL), mybir.dt.float32,
                           kind="ExternalOutput").ap()

    with tile.TileContext(nc) as tc, ExitStack() as ctx:
        _trace_kernel(ctx, tc, out_d, fvt_d, w_d, lin_d, lb_d,
                      ones_d, repeat=repeat, loop=loop)
    nc.compile()
    _MODULES[key] = nc
    return nc


def prepare_in_maps(inputs):
    """Host-side sharding: batch-split fv, pack each shard sub-major as
    [NSUBS, 128, NKT*512] fp16 (one contiguous DMA per sub), replicate
    the packed weights."""
    fv = np.ascontiguousarray(np.asarray(inputs["feature_vector"], np.float32))
    assert fv.shape == (B, F)
    tables = {k: np.asarray(v, np.float32) for k, v in inputs.items()
              if k != "feature_vector"}
    w_pack = np.ascontiguousarray(_build_w_pack(tables), np.float16)
    lw = np.zeros(FP, np.float32)
    lw[:F] = tables["lin_w"][0]
    # -au*au correction rides the linear chain: a derived fv row holds
    # fv_2626^2 and its linear weight is -||A_u||^2
    lw[F + 1] = -float((tables["age_user_w"][0] ** 2).sum())
    lin32 = np.ascontiguousarray(lw.reshape(NKT, 128).T)
    lb = tables["lin_b"].reshape(1, 1)
    ones16 = np.ones((128, 1), np.float16)

    in_maps = []
    for c in range(NCORES):
        fvt = np.zeros((FP, BL), np.float16)
        fvt[:F] = fv[c * BL:(c + 1) * BL].T
        fvt[F + 1] = fv[c * BL:(c + 1) * BL, 2626] ** 2
        # [t*128+p, s*512+c] -> [s, p, t*512+c]
        fvt = np.ascontiguousarray(
            fvt.reshape(NKT, 128, NSUBS, NSUB).transpose(2, 1, 0, 3)
               .reshape(NSUBS, 128, SW))
        in_maps.append({"fvt": fvt, "wpack": w_pack, "lin32": lin32,
                        "linb": lb, "ones16": ones16})
    return in_maps


def kernel(**inputs) -> np.ndarray:
    # Tracing needs the axon NTFF hook, which this environment lacks; make
    # sure a stray BASS_TRACE=1 can't crash the run.
    os.environ["BASS_NEVER_TRACE"] = "1"
    from concourse import bass_utils

    in_maps = prepare_in_maps(inputs)
    nc = get_module()
    try:
        res = bass_utils.run_bass_kernel_spmd(nc, in_maps,
                                              core_ids=list(range(NCORES)))
    except Exception:
        # transient NRT device errors have been observed on this fabric;
        # one retry after a short pause usually succeeds
        import time
        time.sleep(15)
        res = bass_utils.run_bass_kernel_spmd(nc, in_maps,
                                              core_ids=list(range(NCORES)))
    out = np.concatenate([r["out"].reshape(BL) for r in res.results])
    return out.reshape(B, 1).astype(np.float32)


# revision 17
# speedup vs baseline: 1.1769x; 1.0019x over previous
"""Trainium2 Bass kernel for an FFM (field-aware factorization machine) forward pass.

Reference computation (all fp32):
    12 embedding matmuls over column slices of fv [32768, 2668], 15 pairwise
    dot-product cross terms, a linear layer and a sigmoid.

The kernel is DMA-bound: fv is 22 MB/core in fp16 (~66 us at effective HBM
bandwidth), so the design pushes every engine's busy time below the DMA
floor (measured per-op costs for a [128,512] tile: PE matmul 213ns, DVE
TSS 194 / TT 327 / STT ~750, Pool TT ~990 (Pool TSS/STT are unusable:
8us / unsupported), ACT copy ~500):

  * The 12 embeddings are packed as 64-row halves of 128-row weight blocks:
      A = [uu | ui+S3]   tiles 0..7 + an S3 matmul on tile 20
      B = [ti | tu+mu]   tiles 7..20
      D = [mi | au+gu],  E = [R | au+ou]   (tile 20 only)
    where R = au+gu+ou, S3 = ai+gi+oi. Since the cross terms contain
    (tu+mu)*(ui+S3), accumulating S3 into psA's high half merges two
    products into one. 25 block matmuls + 1 reduce matmul per sub.
  * Cross terms are 4 tensor_tensor half-products on drained fp16 tiles:
    uu*R, (ui+S3)*(tu+mu), mi*ti, (au+gu)*(au+ou); the -au*au correction
    rides the linear chain via a host-derived fv row holding fv_2626^2
    with linear weight -||A_u||^2. Products + fold adds run on the DVE,
    emitted at the HEAD of the next sub's program so the in-order DVE
    queue clears the retire path early and the single ones-matmul that
    closes the logit never stalls the PE (routing them through the Pool
    engine or the queue tail measured ~10 us slower).
  * The linear term is split 6/3/12: six K-tiles as M=1 fp16 PE matmuls
    into the logit PSUM (grouped after the blocks), three as ACT copies
    with a per-partition scale, the rest as DVE TSS-multiply + TT-add
    pairs (94+306 ns, cheaper than the fused STT's 752 ns on HW).
  * PSUM is drained to fp16 SBUF on the ACT engine; all DVE operands
    are then 2-byte + SBUF-only, enabling the DVE fast modes.
  * Sub n's reduce + sigmoid retire one sub late (software pipelining)
    so the in-order engine queues never stall on the previous sub's tail.
  * fv is streamed sub-major: one contiguous [128, 21*512] fp16 DMA per
    512-batch sub (host pre-arranges the layout), split across two HWDGE
    rings for bandwidth.

Distribution: data-parallel over the batch dim - each of the 8 cores gets
4096 rows, cast to fp16 host-side (halves HBM traffic; rel err ~4e-3 vs
the 2e-2 gate).
"""

import os
import numpy as np
from contextlib import ExitStack

B, F, D = 32768, 2668, 64
NCORES = 8
BL = B // NCORES          # batch rows per core
NKT = 21                  # feature K-tiles of 128
FP = NKT * 128            # padded feature dim (2688)
NSUB = 512                # matmul moving-dim (one fp32 PSUM bank)
NSUBS = BL // NSUB        # subs per core (8)
SW = NKT * NSUB           # packed width of one sub (10752)

# w_pack column layout: A t0..7 | B t7..20 | S3 | D | E  (128 cols per tile)
A_TILES = tuple(range(0, 8))
B_TILES = tuple(range(7, 21))
AOFF = {t: i * 128 for i, t in enumerate(A_TILES)}
BOFF = {t: (8 + i) * 128 for i, t in enumerate(B_TILES)}
S3OFF, DOFF, EOFF = 22 * 128, 23 * 128, 24 * 128
ZOFF = 25 * 128           # zero block (load-line probe dummies)
WF = 26 * 128

# linear-term K-tile split across engines (tunable)
PE_LIN = int(os.environ.get("FFM_PE_LIN", "6"))
ACT_LIN = int(os.environ.get("FFM_ACT_LIN", "3"))


def _lin_split():
    """Deterministic partition of the 21 K-tiles among PE/ACT/DVE."""
    order = list(range(NKT))
    pe_t = order[0::2][:PE_LIN]
    rest = [t for t in order if t not in pe_t]
    act_t = rest[0::2][:ACT_LIN]
    dve_t = [t for t in rest if t not in act_t]
    return tuple(pe_t), set(act_t), set(dve_t)


PE_T, ACT_T, DVE_T = _lin_split()


def _build_w_pack(inp):
    """Pack the block tables into one [128, WF] array laid out exactly as
    the SBUF weight tile wants it (partition k = row-in-K-tile)."""
    A_u, A_i = inp["age_user_w"], inp["age_item_w"]
    G_u, G_i = inp["gender_user_w"], inp["gender_item_w"]
    O_u, O_i = inp["occupation_user_w"], inp["occupation_item_w"]
    M_u, M_i = inp["movie_user_w"], inp["movie_item_w"]
    U_u, U_i = inp["userid_user_w"], inp["userid_item_w"]
    T_u, T_i = inp["itemid_user_w"], inp["itemid_item_w"]

    WA = np.zeros((FP, 128), np.float32)
    WA[0:943, 0:64] = U_u; WA[0:943, 64:128] = U_i
    WB = np.zeros((FP, 128), np.float32)
    WB[943:2625, 0:64] = T_i
    WB[943:2625, 64:128] = T_u; WB[2649:2668, 64:128] = M_u
    WS3 = np.zeros((FP, 128), np.float32)
    WS3[2626:2627, 64:128] += A_i; WS3[2626:2628, 64:128] += G_i
    WS3[2628:2649, 64:128] += O_i
    WD = np.zeros((FP, 128), np.float32)
    WD[2649:2668, 0:64] = M_i
    WD[2626:2627, 64:128] += A_u; WD[2626:2628, 64:128] += G_u
    WE = np.zeros((FP, 128), np.float32)
    WE[2626:2627, 0:64] += A_u; WE[2626:2628, 0:64] += G_u
    WE[2628:2649, 0:64] += O_u
    WE[2626:2627, 64:128] += A_u; WE[2628:2649, 64:128] += O_u

    w_pack = np.zeros((128, WF), np.float32)
    for t in A_TILES:
        w_pack[:, AOFF[t]:AOFF[t] + 128] = WA[t * 128:(t + 1) * 128]
    for t in B_TILES:
        w_pack[:, BOFF[t]:BOFF[t] + 128] = WB[t * 128:(t + 1) * 128]
    w_pack[:, S3OFF:S3OFF + 128] = WS3[20 * 128:21 * 128]
    w_pack[:, DOFF:DOFF + 128] = WD[20 * 128:21 * 128]
    w_pack[:, EOFF:EOFF + 128] = WE[20 * 128:21 * 128]
    return w_pack


def _trace_kernel(ctx: ExitStack, tc, out_d, fvt_d, w_d, lin_d, lb_d,
                  ones_d, repeat=1, loop=False):
    import concourse.mybir as mybir

    nc = tc.nc
    f32 = mybir.dt.float32
    f16 = mybir.dt.float16
    MUL = mybir.AluOpType.mult
    ADD = mybir.AluOpType.add
    COPY = mybir.ActivationFunctionType.Copy

    wpool = ctx.enter_context(tc.tile_pool(name="wpool", bufs=1))
    w_sb = wpool.tile([128, WF], f16, name="w_sb")
    nc.sync.dma_start(w_sb[:, 0:128], w_d[:, 0:128])
    nc.sync.dma_start(w_sb[:, 128:WF], w_d[:, 128:WF])
    lin_sb = wpool.tile([128, NKT], f32, name="lin_sb")
    nc.sync.dma_start(lin_sb[:], lin_d[:])
    lin16_sb = wpool.tile([128, NKT], f16, name="lin16_sb")
    nc.gpsimd.dma_start(lin16_sb[:], lin_d[:])  # casting DMA (f32 -> f16)
    lb_sb = wpool.tile([1, 1], f32, name="lb_sb")
    nc.sync.dma_start(lb_sb[:], lb_d[:])
    ones_sb = wpool.tile([128, 1], f16, name="ones_sb")
    nc.sync.dma_start(ones_sb[:], ones_d[:])

    fpool = ctx.enter_context(tc.tile_pool(
        name="fpool", bufs=int(os.environ.get("FFM_FBUFS", "4"))))
    pspool = ctx.enter_context(tc.tile_pool(name="pspool", bufs=1, space="PSUM"))
    spool = ctx.enter_context(tc.tile_pool(name="spool", bufs=3))
    opool = ctx.enter_context(tc.tile_pool(name="opool", bufs=2))
    out_eng = {"sync": nc.sync, "scalar": nc.scalar,
               "gpsimd": nc.gpsimd}[os.environ.get("FFM_OUTDMA", "sync")]
    lin_stt = os.environ.get("FFM_LIN_STT", "0") == "1"
    pool_prod = int(os.environ.get("FFM_POOL_PROD", "0"))
    pool_folds = int(os.environ.get("FFM_POOL_FOLDS", "0"))
    reduces = int(os.environ.get("FFM_REDUCES", "1"))
    # load-line probes: extra no-op work to measure an engine's marginal cost
    xpe = int(os.environ.get("FFM_XPE", "0"))
    xdve = int(os.environ.get("FFM_XDVE", "0"))
    # software-pipeline depth: subs between drain and reduce/sigmoid retire.
    # PSUM budget (8 banks) forces psB single-buffered at depth 2:
    # psA 2 + psB (2|1) + psD 1 + psE 1 + logit (depth+1) = 8 either way.
    depth = int(os.environ.get("FFM_DEPTH", "2"))
    psb_bufs = 2 if depth == 1 else 1
    logit_bufs = depth + 1

    HALF = SW // 2  # ring-split point of the packed sub row

    def _products(p):
        """Cross products + folds for a sub whose drains landed a sub ago."""
        sid = p["sid"]
        dA, dB, dD, dE = p["dA"], p["dB"], p["dD"], p["dE"]
        st1 = spool.tile([128, NSUB], f16, tag="st1", name=f"st1_{sid}")
        st2 = spool.tile([128, NSUB], f16, tag="st2", name=f"st2_{sid}")
        prods = [
            (st1, slice(0, 64), dA, dE),        # uu*R
            (st1, slice(64, 128), dA, dB),      # (ui+S3)*(tu+mu)
            (st2, slice(0, 64), dD, dB),        # mi*ti
            (st2, slice(64, 128), dD, dE),      # (au+gu)*(au+ou) [-au^2 in lin]
        ]
        for j, (dst, sl, x, y) in enumerate(prods):
            eng = nc.gpsimd if j < pool_prod else nc.vector
            eng.tensor_mul(dst[sl], x[sl], y[sl])
        # fold ACT-lin tmp tiles into the DVE accumulators
        accs = [a for a in (p["acc0"], p["acc1"]) if a is not None]
        srcs = [st1, st2] + accs
        for i, tmp in enumerate(p["atmps"]):
            if accs:
                tgt = accs[i % len(accs)]
                nc.vector.tensor_add(tgt[:], tgt[:], tmp[:])
            else:
                srcs.append(tmp)
        # fold source pairs so fewer PE reduce matmuls are needed
        nf = 0
        while len(srcs) > max(reduces, 1):
            a = srcs.pop(0); b = srcs.pop(0)
            eng = nc.gpsimd if nf < pool_folds else nc.vector
            eng.tensor_add(a[:], a[:], b[:])
            srcs.append(a)
            nf += 1
        p["red"] = tuple(srcs)

    def _finish(p):
        """Reduce + sigmoid + store for a sub whose products are complete."""
        red, col, logit = p["red"], p["col"], p["logit"]
        for j, srct in enumerate(red):
            nc.tensor.matmul(logit[:], ones_sb[:], srct[:],
                             start=(j == 0 and not PE_T),
                             stop=(j == len(red) - 1))
        out_sb = opool.tile([1, NSUB], f32, tag="out", name=f"out_{col}")
        nc.scalar.activation(out_sb[:], logit[:],
                             mybir.ActivationFunctionType.Sigmoid,
                             bias=lb_sb[0:1, 0:1], scale=1.0)
        out_eng.dma_start(out_d[0:1, col:col + NSUB], out_sb[:])

    def _body(rep, passes=1):
        pend = []  # software-pipeline: sub n retires during sub n+1
        for sp in range(passes * NSUBS):
            s = sp % NSUBS
            big = fpool.tile([128, SW], f16, tag="fv", name=f"fv_{rep}_{sp}")
            nc.sync.dma_start(big[:, 0:HALF], fvt_d[s][:, 0:HALF])
            nc.scalar.dma_start(big[:, HALF:SW], fvt_d[s][:, HALF:SW])
            # retire path of the previous sub FIRST: its products/folds land
            # at the head of the in-order DVE queue (its drains are already
            # done), so the PE reduce emitted at this sub's tail never stalls
            if pend:
                _products(pend[-1])

            def rhs(t):
                return big[:, t * NSUB:(t + 1) * NSUB]
            sid = f"{rep}_{sp}"
            psA = pspool.tile([128, NSUB], f32, tag="psA", bufs=2,
                              name=f"psA_{sid}")
            psB = pspool.tile([128, NSUB], f32, tag="psB", bufs=psb_bufs,
                              name=f"psB_{sid}")
            logit = pspool.tile([1, NSUB], f32, tag="logit", bufs=logit_bufs,
                                name=f"logit_{sid}")
            psD = pspool.tile([128, NSUB], f32, tag="psD", name=f"psD_{sid}")
            psE = pspool.tile([128, NSUB], f32, tag="psE", name=f"psE_{sid}")
            accs = [None, None]   # DVE parities (hide RAW latency)
            atmps = []            # ACT-lin tmp tiles
            ndve = 0
            for t in range(NKT):
                if t in A_TILES:
                    nc.tensor.matmul(psA[:], w_sb[:, AOFF[t]:AOFF[t] + 128],
                                     rhs(t), start=(t == 0), stop=False)
                if t in B_TILES:
                    nc.tensor.matmul(psB[:], w_sb[:, BOFF[t]:BOFF[t] + 128],
                                     rhs(t), start=(t == 7), stop=(t == 20))
                if t == 10:
                    for _x in range(xpe):   # zero-weight: psB unchanged
                        nc.tensor.matmul(psB[:], w_sb[:, ZOFF:ZOFF + 128],
                                         rhs(t), start=False, stop=False)
                    if xdve:
                        xsc = spool.tile([128, NSUB], f16, tag="xdve",
                                         name=f"xdve_{sid}")
                        for _x in range(xdve):
                            nc.vector.tensor_mul(xsc[:], rhs(0), rhs(1))
                if t == 20:
                    nc.tensor.matmul(psA[:], w_sb[:, S3OFF:S3OFF + 128],
                                     rhs(t), start=False, stop=True)
                    nc.tensor.matmul(psD[:], w_sb[:, DOFF:DOFF + 128],
                                     rhs(t), start=True, stop=True)
                    nc.tensor.matmul(psE[:], w_sb[:, EOFF:EOFF + 128],
                                     rhs(t), start=True, stop=True)
                # linear term: per-partition-scalar multiply-accumulate,
                # split across PE (M=1 matmuls grouped after the blocks),
                # ACT (scaled copies) and DVE (TSS+add pairs)
                wcol = lin_sb[:, t:t + 1]
                if t in PE_T:
                    pass  # emitted below, grouped with the reduce
                elif t in ACT_T:
                    at = spool.tile([128, NSUB], f16, tag=f"atmp{len(atmps)}",
                                    name=f"atmp{len(atmps)}_{sid}")
                    nc.scalar.activation(at[:], rhs(t), COPY, scale=wcol)
                    atmps.append(at)
                else:
                    par = ndve % 2
                    ndve += 1
                    if accs[par] is None:
                        at = spool.tile([128, NSUB], f16, tag=f"acc{par}",
                                        name=f"acc{par}_{sid}")
                        nc.vector.tensor_single_scalar(at[:], rhs(t),
                                                       wcol, MUL)
                        accs[par] = at
                    elif lin_stt:
                        nc.vector.scalar_tensor_tensor(
                            accs[par][:], rhs(t), wcol, accs[par][:],
                            MUL, ADD)
                    else:
                        tmp = spool.tile([128, NSUB], f16, tag=f"tmp{par}",
                                         name=f"tmp{par}_{sid}_{t}")
                        nc.vector.tensor_single_scalar(tmp[:], rhs(t),
                                                       wcol, MUL)
                        nc.vector.tensor_add(accs[par][:], accs[par][:],
                                             tmp[:])
            # linear-term M=1 matmuls, grouped so the PE switches from
            # 128-row to 1-row output tiles only once per sub
            for j, t in enumerate(PE_T):
                nc.tensor.matmul(logit[:], lin16_sb[:, t:t + 1],
                                 rhs(t), start=(j == 0), stop=False)
            # drains: PSUM -> fp16 SBUF on the ACT engine, in PE completion
            # order (B stops first, then A/D/E at t20)
            dB = spool.tile([128, NSUB], f16, tag="dB", name=f"dB_{sid}")
            nc.scalar.copy(dB[:], psB[:])
            dA = spool.tile([128, NSUB], f16, tag="dA", name=f"dA_{sid}")
            nc.scalar.copy(dA[:], psA[:])
            dD = spool.tile([128, NSUB], f16, tag="dD", name=f"dD_{sid}")
            nc.scalar.copy(dD[:], psD[:])
            dE = spool.tile([128, NSUB], f16, tag="dE", name=f"dE_{sid}")
            nc.scalar.copy(dE[:], psE[:])
            # retire older subs: reduce + sigmoid `depth` subs behind (their
            # products were emitted at the head of the following sub)
            while len(pend) >= depth:
                _finish(pend.pop(0))
            pend.append({"sid": sid, "dA": dA, "dB": dB, "dD": dD,
                         "dE": dE, "acc0": accs[0], "acc1": accs[1],
                         "atmps": atmps, "logit": logit, "col": s * NSUB})
        _products(pend[-1])
        for p in pend:
            _finish(p)

    if loop and repeat > 1:
        # benchmarking mode: run the identical body `repeat` times inside one
        # NEFF via a hardware loop, multiple passes per iteration so the
        # software pipeline flows across pass boundaries.
        if repeat % 16 == 0:
            with tc.For_i(0, repeat // 16, 1):
                _body(0, passes=16)
        elif repeat % 8 == 0:
            with tc.For_i(0, repeat // 8, 1):
                _body(0, passes=8)
        elif repeat % 4 == 0:
            with tc.For_i(0, repeat // 4, 1):
                _body(0, passes=4)
        elif repeat % 2 == 0:
            with tc.For_i(0, repeat // 2, 1):
                _body(0, passes=2)
        else:
            with tc.For_i(0, repeat, 1):
                _body(0)
    else:
        for rep in range(repeat):
            _body(rep)


_MODULES = {}


def get_module(repeat=1, loop=False):
    """Build (once per config) and return the compiled Bass module."""
    key = (repeat, loop)
    if key in _MODULES:
        return _MODULES[key]

    import concourse.bacc as bacc
    import concourse.tile as tile
    import concourse.mybir as mybir

    nc = bacc.Bacc("TRN2", debug=False, enable_asserts=False,
                   num_devices=NCORES)
    fvt_d = nc.dram_tensor("fvt", (NSUBS, 128, SW), mybir.dt.float16,
                           kind="ExternalInput").ap()
    w_d = nc.dram_tensor("wpack", (128, WF), mybir.dt.float16,
                         kind="ExternalInput").ap()
    lin_d = nc.dram_tensor("lin32", (128, NKT), mybir.dt.float32,
                           kind="ExternalInput").ap()
    lb_d = nc.dram_tensor("linb", (1, 1), mybir.dt.float32,
                          kind="ExternalInput").ap()
    ones_d = nc.dram_tensor("ones16", (128, 1), mybir.dt.float16,
                            kind="ExternalInput").ap()
    out_d = nc.dram_tensor("out", (1, BL), mybir.dt.float32,
                           kind="ExternalOutput").ap()

    with tile.TileContext(nc) as tc, ExitStack() as ctx:
        _trace_kernel(ctx, tc, out_d, fvt_d, w_d, lin_d, lb_d,
                      ones_d, repeat=repeat, loop=loop)
    nc.compile()
    _MODULES[key] = nc
    return nc


def prepare_in_maps(inputs):
    """Host-side sharding: batch-split fv, pack each shard sub-major as
    [NSUBS, 128, NKT*512] fp16 (one contiguous DMA per sub), replicate
    the packed weights."""
    fv = np.ascontiguousarray(np.asarray(inputs["feature_vector"], np.float32))
    assert fv.shape == (B, F)
    tables = {k: np.asarray(v, np.float32) for k, v in inputs.items()
              if k != "feature_vector"}
    w_pack = np.ascontiguousarray(_build_w_pack(tables), np.float16)
    lw = np.zeros(FP, np.float32)
    lw[:F] = tables["lin_w"][0]
    # -au*au correction rides the linear chain: a derived fv row holds
    # fv_2626^2 and its linear weight is -||A_u||^2
    lw[F + 1] = -float((tables["age_user_w"][0] ** 2).sum())
    lin32 = np.ascontiguousarray(lw.reshape(NKT, 128).T)
    lb = tables["lin_b"].reshape(1, 1)
    ones16 = np.ones((128, 1), np.float16)

    in_maps = []
    for c in range(NCORES):
        fvt = np.zeros((FP, BL), np.float16)
        fvt[:F] = fv[c * BL:(c + 1) * BL].T
        fvt[F + 1] = fv[c * BL:(c + 1) * BL, 2626] ** 2
        # [t*128+p, s*512+c] -> [s, p, t*512+c]
        fvt = np.ascontiguousarray(
            fvt.reshape(NKT, 128, NSUBS, NSUB).transpose(2, 1, 0, 3)
               .reshape(NSUBS, 128, SW))
        in_maps.append({"fvt": fvt, "wpack": w_pack, "lin32": lin32,
                        "linb": lb, "ones16": ones16})
    return in_maps


def kernel(**inputs) -> np.ndarray:
    # Tracing needs the axon NTFF hook, which this environment lacks; make
    # sure a stray BASS_TRACE=1 can't crash the run.
    os.environ["BASS_NEVER_TRACE"] = "1"
    from concourse import bass_utils

    in_maps = prepare_in_maps(inputs)
    nc = get_module()
    try:
        res = bass_utils.run_bass_kernel_spmd(nc, in_maps,
                                              core_ids=list(range(NCORES)))
    except Exception:
        # transient NRT device errors have been observed on this fabric;
        # one retry after a short pause usually succeeds
        import time
        time.sleep(15)
        res = bass_utils.run_bass_kernel_spmd(nc, in_maps,
                                              core_ids=list(range(NCORES)))
    out = np.concatenate([r["out"].reshape(BL) for r in res.results])
    return out.reshape(B, 1).astype(np.float32)
